# revision 28
# baseline (speedup 1.0000x reference)
"""Trainium2 Bass kernel for nn_Net_69114613727316 (RGCN message passing).

Self-contained: kernel(**inputs) -> np.ndarray [100000] float32.

Math (exploiting num_bases=1): w[r] = att[r,0] * basis, so
    agg_d = ((sum_e v_e * x[src_e]) @ basis),  v_e = att_l[etype_e]/max(cnt_d,1)
    out = relu(agg + x @ root + bias)
Per-edge GEMMs collapse into a weighted scatter-add (one-hot matmuls into
PSUM per 256-dst "superband" window) plus one dense GEMM per window.

Sharding: node space is permuted so core c owns rows
[c*25088, (c+1)*25088) = [12544 var slots | 12544 con slots].  Each
layer's full x (bf16) is replicated via piece-wise AllGather so any core
can gather arbitrary src rows; each core computes only its own rows.

v6 (from v5):
  * degree-balanced node->slot packing per core: per-(superband, chunk)
    edge counts equalized so every (sb, chunk) needs exactly 1 128-edge
    tile; zero-in-degree nodes concentrated in dedicated superbands
    whose scatter is skipped entirely
  * stripe-mined gathers: one dma_gather per (24..26-superband stripe,
    src chunk) -> 32 calls/layer instead of 72
  * piece-wise AllGather (4 pieces per layer, launched as each stripe's
    rows finish) overlapping collective latency under compute
  * per-superband small stores (outg/rowg/xT) to cut SBUF pressure
  * phase-A input MLPs processed in 256-column pairs
"""
import numpy as np
import ml_dtypes

import concourse.bass as bass
import concourse.bacc as bacc
import concourse.tile as tile
import concourse.mybir as mybir
from concourse import library_config

F32 = mybir.dt.float32
BF16 = mybir.dt.bfloat16
I16 = mybir.dt.int16
D = 128
P = 128
SB = 256                      # superband width (dst window)

N_VAR = 100000
N_CON = 100000
N_EDGES = 640000
N_CORES = 8

NV_CORE = N_VAR // N_CORES          # 12500
NC_CORE = N_CON // N_CORES          # 12500
NV_SLOT = ((NV_CORE + P - 1) // P) * P   # 12544
NC_SLOT = ((NC_CORE + P - 1) // P) * P   # 12544
PER_CORE = NV_SLOT + NC_SLOT        # 25088
N_PAD = N_CORES * PER_CORE          # 200704
NSB = PER_CORE // SB                # 98 superbands
NSB_VAR = NV_SLOT // SB             # 49 (sbs 0..48 var, 49..97 con)

GS = [20, 20, 20, 20, 18]           # gather stripe sizes (superbands)
SUBS = [[10, 10], [10, 10], [10, 10], [10, 10], [9, 9]]  # og sub-blocks
MSUB = max(max(s) for s in SUBS)    # max sub width (superbands)
NPIECE = 8                          # x replication pieces (slot ranges)
PIECE_ROWS = PER_CORE // NPIECE     # 3136 rows per piece per core

BF = ml_dtypes.bfloat16


def _bf(a):
    return np.asarray(a, np.float32).astype(BF)


def _pack_nodes(deg_chunk, nsb_zero=1):
    """Assign node-local-ids (0..n-1) to slots within 49 superbands.

    deg_chunk: [n, 8] per-chunk in-degree. Returns slot_of_node [n].
    Last `nsb_zero` superbands get only zero-degree nodes (if enough),
    remaining nodes snake-dealt by total degree into the other sbs.
    """
    n = deg_chunk.shape[0]
    nsb = 49
    tot = deg_chunk.sum(axis=1)
    order = np.argsort(-tot, kind="stable")
    zero_ids = order[tot[order] == 0]
    nz_ids = order[tot[order] > 0]
    slot = np.full(n, -1, np.int64)
    # fill zero sbs from the back with zero-degree nodes
    zcap = nsb_zero * SB
    if len(zero_ids) < zcap:
        # not enough true-zero nodes: pull lowest-degree nodes too
        extra = nz_ids[len(nz_ids) - (zcap - len(zero_ids)):]
        nz_ids = nz_ids[:len(nz_ids) - (zcap - len(zero_ids))]
        zfill = np.concatenate([zero_ids, extra])
    else:
        zfill = zero_ids[:zcap]
        nz_ids = np.concatenate([nz_ids, zero_ids[zcap:]])
    slot[zfill] = (nsb - nsb_zero) * SB + np.arange(zcap)
    # greedy chunk-aware packing of nonzero nodes (degree-desc): place each
    # node in the sb minimizing the resulting max per-chunk count
    nact = nsb - nsb_zero
    k = len(nz_ids)
    cnt = np.zeros((nact, N_CORES), np.int64)
    used = np.zeros(nact, np.int64)
    sb_of = np.zeros(k, np.int64)
    degs = deg_chunk[nz_ids]
    for i in range(k):
        dn = degs[i]
        score = (cnt + dn).max(axis=1) * 1024 + used
        score[used >= SB] = 1 << 60
        s = int(np.argmin(score))
        sb_of[i] = s
        cnt[s] += dn
        used[s] += 1
    # rank within sb = order of assignment
    rank = np.zeros(k, np.int64)
    srt = np.argsort(sb_of, kind="stable")
    sb_sorted = sb_of[srt]
    starts = np.searchsorted(sb_sorted, np.arange(nact))
    ends = np.searchsorted(sb_sorted, np.arange(nact), side="right")
    for s in range(nact):
        sel = srt[starts[s]:ends[s]]
        rank[sel] = np.arange(ends[s] - starts[s])
    assert (rank < SB).all(), "superband overflow in packing"
    slot[nz_ids] = sb_of * SB + rank
    assert (slot >= 0).all()
    return slot


def _preprocess(inputs):
    vf = np.ascontiguousarray(np.asarray(inputs["var_node_features"], np.float32))
    cf = np.ascontiguousarray(np.asarray(inputs["con_node_features"], np.float32))
    ei = np.asarray(inputs["edge_index"])
    et = np.asarray(inputs["edge_types"]).astype(np.int64)
    assert (np.asarray(inputs["assoc_var"]) == np.arange(N_VAR)).all()
    assert (np.asarray(inputs["assoc_con"]) == N_VAR + np.arange(N_CON)).all()

    src = ei[0].astype(np.int64)
    dst = ei[1].astype(np.int64)

    # node -> (core, kind, local id): var v -> core v//NV_CORE, local v%NV_CORE
    node = np.arange(N_VAR + N_CON)
    isv_n = node < N_VAR
    core_n = np.where(isv_n, node // NV_CORE, (node - N_VAR) // NC_CORE)
    loc_n = np.where(isv_n, node % NV_CORE, (node - N_VAR) % NC_CORE)

    # per-node per-src-chunk in-degree (chunk = src core after permute —
    # but chunk of an edge depends on the SRC node's core, known already)
    src_core = np.where(src < N_VAR, src // NV_CORE, (src - N_VAR) // NC_CORE)
    # in-degree per (dst node, src core)
    degc = np.zeros((N_VAR + N_CON, N_CORES), np.int64)
    np.add.at(degc, (dst, src_core), 1)

    # pack per core: var + con separately; iterate with src-piece-aware
    # degree vectors (gather chunk = slot-range piece) and keep the best
    def pack_all(dc):
        slot = np.zeros(N_VAR + N_CON, np.int64)
        for c in range(N_CORES):
            vids = np.arange(c * NV_CORE, (c + 1) * NV_CORE)
            slot[vids] = _pack_nodes(dc[vids], nsb_zero=1)
            cids = N_VAR + np.arange(c * NC_CORE, (c + 1) * NC_CORE)
            slot[cids] = NV_SLOT + _pack_nodes(dc[cids], nsb_zero=1)
        return slot

    def t2_of(slot):
        pm = core_n * PER_CORE + slot
        pc = (pm[src] % PER_CORE) // PIECE_ROWS
        sb_ = (pm[dst] % PER_CORE) // SB
        sg = ((pm[dst] // PER_CORE) * NSB + sb_) * NPIECE + pc
        cn = np.bincount(sg, minlength=N_CORES * NSB * NPIECE).reshape(
            N_CORES, NSB, NPIECE)
        return int(np.ceil(cn.max(axis=0) / P).sum())

    slot_of_node = pack_all(degc)
    best = (t2_of(slot_of_node), slot_of_node)
    cur = slot_of_node
    for _ in range(2):
        pm = core_n * PER_CORE + cur
        piece_lbl = (pm[src] % PER_CORE) // PIECE_ROWS
        degc2 = np.zeros((N_VAR + N_CON, NPIECE), np.int64)
        np.add.at(degc2, (dst, piece_lbl), 1)
        cur = pack_all(degc2)
        t2v = t2_of(cur)
        if t2v < best[0]:
            best = (t2v, cur)
    slot_of_node = best[1]

    perm = core_n * PER_CORE + slot_of_node    # node -> padded row
    psrc = perm[src]
    pdst = perm[dst]
    cnt = np.bincount(pdst, minlength=N_PAD).astype(np.float32)
    invc = 1.0 / np.maximum(cnt, 1.0)
    atts = [np.asarray(inputs[f"att{l}"], np.float32)[:, 0] for l in (1, 2, 3)]

    core = pdst // PER_CORE
    sb = (pdst % PER_CORE) // SB          # superband 0..97
    dl2 = (pdst % SB).astype(np.float32)  # dst offset in window, 0..255
    # gather chunk = slot-range piece p: x_p[p] holds rows [3136p,3136(p+1))
    # of every core, laid out [core, row-in-piece]
    chunk = (psrc % PER_CORE) // PIECE_ROWS          # src piece 0..7
    lsrc = ((psrc // PER_CORE) * PIECE_ROWS
            + (psrc % PER_CORE) % PIECE_ROWS).astype(np.int16)

    # tiles per (superband, chunk): shared across cores (max)
    seg = (core * NSB + sb) * N_CORES + chunk
    counts = np.bincount(seg, minlength=N_CORES * NSB * N_CORES).reshape(
        N_CORES, NSB, N_CORES)
    tpb2 = np.ceil(counts.max(axis=0) / P).astype(np.int64)  # [NSB, CHUNKS]

    # canonical tile-column order: stripe G -> chunk c -> sb in G -> tiles
    assert sum(GS) == NSB
    groups = []
    s0 = 0
    for gsz in GS:
        groups.append(list(range(s0, s0 + gsz)))
        s0 += gsz
    colmap = {}
    ncol = 0
    for g, sbs in enumerate(groups):
        for c in range(N_CORES):
            for s in sbs:
                colmap[(s, c)] = ncol
                ncol += int(tpb2[s, c])
    T2 = ncol

    # rank of each edge within its (core, sb, chunk) run
    order = np.argsort(seg, kind="stable")
    seg_s = seg[order]
    run_starts = np.concatenate(
        [[0], np.cumsum(np.bincount(seg_s, minlength=seg.max() + 1))[:-1]])
    rank = np.arange(len(order)) - run_starts[seg_s]
    core_s = core[order]
    col0 = np.array([colmap[(s, c)] for s, c in
                     zip(sb[order].tolist(), chunk[order].tolist())])
    tcol = col0 + rank // P
    prow = rank % P

    lsrc_arr = np.zeros((N_CORES, P, T2), np.int16)
    dl_arr = np.zeros((N_CORES, P, T2), np.float32)
    vw_arr = np.zeros((3, N_CORES, P, T2), np.float32)
    lsrc_arr[core_s, prow, tcol] = lsrc[order]
    dl_arr[core_s, prow, tcol] = dl2[order]
    for li in range(3):
        vw_arr[li, core_s, prow, tcol] = atts[li][et[order]] * invc[pdst[order]]

    # int16 idxs in dma_gather layout: for linear j in a call, value at
    # partition 16*r + j%16 (replicated r=0..7), column tcol*8 + prow//16.
    idx16 = np.zeros((N_CORES, P, T2 * 8), np.int16)
    tc_all = np.repeat(np.arange(T2), P)
    pr_all = np.tile(np.arange(P), T2)
    cols = tc_all * 8 + pr_all // 16
    rows_ = pr_all % 16
    for cidx in range(N_CORES):
        vals = lsrc_arr[cidx, pr_all, tc_all]
        for r in range(8):
            idx16[cidx, 16 * r + rows_, cols] = vals

    # per-(stripe,chunk) call shapes
    call_tiles = []
    for g, sbs in enumerate(groups):
        call_tiles.append(
            [int(sum(int(tpb2[s, c]) for s in sbs)) for c in range(N_CORES)])

    iota256 = np.broadcast_to(
        np.arange(SB, dtype=np.float32), (P, SB)).astype(BF)
    ident = np.eye(P, dtype=np.float32).astype(BF)

    # per-core permuted features (staged per slot)
    in_maps = []
    for c in range(N_CORES):
        vfeat = np.zeros((NV_SLOT, 2), np.float32)
        vids = np.arange(c * NV_CORE, (c + 1) * NV_CORE)
        vfeat[slot_of_node[vids]] = vf[vids]
        cfeat = np.zeros((NC_SLOT, 2), np.float32)
        cids = np.arange(c * NC_CORE, (c + 1) * NC_CORE)
        cfeat[slot_of_node[N_VAR + cids] - NV_SLOT] = cf[cids]
        m = {
            "vfeatT": _bf(vfeat.T),
            "cfeatT": _bf(cfeat.T),
            "idx16": idx16[c],
            "dl2": dl_arr[c],
            "vw1": vw_arr[0, c],
            "vw2": vw_arr[1, c],
            "vw3": vw_arr[2, c],
            "iota256": iota256,
            "ident": ident,
            "fc1_w": _bf(inputs["fc1_w"]),
            "fc1_b": np.asarray(inputs["fc1_b"], np.float32),
            "fc4_w": _bf(inputs["fc4_w"]),
            "fc4_b": np.broadcast_to(
                np.asarray(inputs["fc4_b"], np.float32).reshape(1, 1),
                (P, 1)).copy(),
        }
        for t in ("var", "con"):
            m[f"{t}_w1"] = _bf(inputs[f"{t}_w1"])
            m[f"{t}_b1"] = np.asarray(inputs[f"{t}_b1"], np.float32)
            m[f"{t}_w2"] = _bf(inputs[f"{t}_w2"])
            m[f"{t}_b2"] = np.asarray(inputs[f"{t}_b2"], np.float32)
        for l in (1, 2, 3):
            m[f"basis{l}"] = _bf(np.asarray(inputs[f"basis{l}"],
                                            np.float32).reshape(D, D))
            m[f"root{l}"] = _bf(inputs[f"root{l}"])
            m[f"bias{l}"] = np.asarray(inputs[f"bias{l}"], np.float32)
        in_maps.append(m)
    # var slot -> global var id map for output assembly
    var_slot = slot_of_node[:N_VAR]    # slot within core var block
    key = tuple(int(t) for t in tpb2.reshape(-1))
    return in_maps, tpb2, groups, call_tiles, T2, var_slot, key


def _build_program(tpb2, groups, call_tiles, T2):
    nc = bacc.Bacc("TRN2", target_bir_lowering=False, debug=False,
                   num_devices=N_CORES, num_swdge_queues=4)

    def inp(name, shape, dtype=BF16):
        return nc.dram_tensor(name, shape, dtype, kind="ExternalInput")

    vfeatT = inp("vfeatT", [2, NV_SLOT])
    cfeatT = inp("cfeatT", [2, NC_SLOT])
    idx16 = inp("idx16", [P, T2 * 8], I16)
    dl2 = inp("dl2", [P, T2], F32)
    vws = {l: inp(f"vw{l}", [P, T2], F32) for l in (1, 2, 3)}
    iota256 = inp("iota256", [P, SB])
    identt = inp("ident", [P, P])
    mlp_w = {}
    for t in ("var", "con"):
        mlp_w[t] = (inp(f"{t}_w1", [2, D]), inp(f"{t}_b1", [D], F32),
                    inp(f"{t}_w2", [D, D]), inp(f"{t}_b2", [D], F32))
    rg_w = {l: (inp(f"basis{l}", [D, D]), inp(f"root{l}", [D, D]),
                inp(f"bias{l}", [D], F32)) for l in (1, 2, 3)}
    fc1_w = inp("fc1_w", [4 * D, D])
    fc1_b = inp("fc1_b", [D], F32)
    fc4_w = inp("fc4_w", [D, 1])
    fc4_b = inp("fc4_b", [P, 1], F32)
    y_out = nc.dram_tensor("y_out", [NV_SLOT], F32, kind="ExternalOutput")

    x_p = [[nc.dram_tensor(f"x{i}_p{p}", [N_CORES * PIECE_ROWS, D], BF16,
                           kind="Internal", addr_space="Shared")
            for p in range(NPIECE)] for i in range(3)]
    ag_in = [nc.dram_tensor(f"ag_in{i}", [PER_CORE, D], BF16, kind="Internal")
             for i in range(3)]
    xT_own = [nc.dram_tensor(f"xT{i}_own", [D, PER_CORE], BF16,
                             kind="Internal") for i in range(4)]

    rgroups = [list(range(N_CORES))]
    grp_cstart = []
    grp_base = []
    base = 0
    for g in range(len(groups)):
        cst = []
        off = 0
        for c in range(N_CORES):
            cst.append(off)
            off += call_tiles[g][c]
        grp_cstart.append(cst)
        grp_base.append(base)
        base += off
    max_grp_tiles = max(sum(ct) for ct in call_tiles)

    with tile.TileContext(nc) as tc:
        with tc.tile_pool(name="wp", bufs=1) as wp:
            nc.gpsimd.load_library(library_config.mlp)
            iota_t = wp.tile([P, SB], BF16, name="iota_t")
            nc.sync.dma_start(iota_t[:], iota256[:])
            ident = wp.tile([P, P], BF16, name="ident")
            nc.sync.dma_start(ident[:], identt[:])

            idx_sb = wp.tile([P, T2 * 8], I16, name="idx_sb")
            nc.sync.dma_start(idx_sb[:], idx16[:])
            dl_sb = wp.tile([P, T2], F32, name="dl_sb")
            nc.sync.dma_start(dl_sb[:], dl2[:])
            vw_sb = {}
            for l in (1, 2, 3):
                vw_sb[l] = wp.tile([P, T2], F32, name=f"vw_sb{l}")
                nc.sync.dma_start(vw_sb[l][:], vws[l][:])

            # ---------- phase A: input MLPs -> x0 (pairs of bands) ----------
            with tc.tile_pool(name="pa_f", bufs=1) as fp, \
                 tc.tile_pool(name="pa_sb", bufs=3) as sp, \
                 tc.tile_pool(name="pa_o", bufs=4) as op_a, \
                 tc.tile_pool(name="pa_pst", bufs=2, space="PSUM") as pp_t, \
                 tc.tile_pool(name="pa_ps", bufs=2, space="PSUM") as pp:
                pa_pieces = 0
                for ttype, featT, slot0, nslot in (
                    ("var", vfeatT, 0, NV_SLOT),
                    ("con", cfeatT, NV_SLOT, NC_SLOT),
                ):
                    w1, b1, w2, b2 = mlp_w[ttype]
                    w1s = wp.tile([2, D], BF16, name=f"w1s_{ttype}")
                    nc.sync.dma_start(w1s[:], w1[:])
                    b1s = wp.tile([P, 1], F32, name=f"b1s_{ttype}")
                    nc.sync.dma_start(b1s[:], b1.rearrange("(p one) -> p one", one=1))
                    w2s = wp.tile([D, D], BF16, name=f"w2s_{ttype}")
                    nc.sync.dma_start(w2s[:], w2[:])
                    b2s = wp.tile([P, 1], F32, name=f"b2s_{ttype}")
                    nc.sync.dma_start(b2s[:], b2.rearrange("(p one) -> p one", one=1))
                    ftT = fp.tile([2, nslot], BF16, name=f"ftT_{ttype}")
                    nc.sync.dma_start(ftT[:], featT[:])
                    npairs = nslot // SB
                    for j in range(npairs):
                        p1 = pp.tile([P, SB], F32, name="p1", space="PSUM")
                        nc.tensor.matmul(p1[:], lhsT=w1s[:],
                                         rhs=ftT[:, j * SB:(j + 1) * SB],
                                         start=True, stop=True)
                        h1 = sp.tile([P, SB], BF16, name="h1")
                        nc.scalar.activation(h1[:], p1[:],
                                             mybir.ActivationFunctionType.Relu,
                                             bias=b1s[:, :1])
                        p2 = pp.tile([P, SB], F32, name="p2", space="PSUM")
                        nc.tensor.matmul(p2[:], lhsT=w2s[:], rhs=h1[:],
                                         start=True, stop=True)
                        xTg = op_a.tile([P, SB], BF16, name="xTg")
                        nc.vector.tensor_scalar(
                            xTg[:], p2[:],
                            b2s[:, :1], None, op0=mybir.AluOpType.add)
                        rowg = op_a.tile([P, SB], BF16, name="rowg")
                        for h in range(2):
                            tp = pp_t.tile([P, P], BF16, name="tp",
                                           space="PSUM")
                            nc.tensor.transpose(
                                tp[:], xTg[:, h * P:(h + 1) * P], ident[:])
                            nc.scalar.activation(
                                rowg[:, h * P:(h + 1) * P], tp[:],
                                mybir.ActivationFunctionType.Copy)
                        col0 = slot0 + j * SB
                        nc.sync.dma_start(
                            xT_own[0][:, col0:col0 + SB], xTg[:])
                        nc.sync.dma_start(
                            ag_in[0][col0:col0 + SB, :]
                            .rearrange("(t p) d -> p t d", p=P),
                            rowg[:].rearrange("p (t d) -> p t d", d=D))
                        while (pa_pieces < NPIECE
                               and col0 + SB >= (pa_pieces + 1) * PIECE_ROWS):
                            pz = pa_pieces
                            nc.gpsimd.collective_compute(
                                "AllGather", mybir.AluOpType.bypass,
                                replica_groups=rgroups,
                                ins=[ag_in[0][pz * PIECE_ROWS:
                                              (pz + 1) * PIECE_ROWS, :]],
                                outs=[x_p[0][pz][:]])
                            pa_pieces += 1

            # head-MLP weights (consumed inside layer 3)
            fc1c = []
            for hl in range(4):
                t = wp.tile([D, D], BF16, name=f"fc1c{hl}")
                nc.sync.dma_start(t[:], fc1_w[hl * D:(hl + 1) * D, :])
                fc1c.append(t)
            fb1 = wp.tile([P, 1], F32, name="fb1")
            nc.sync.dma_start(fb1[:], fc1_b.rearrange("(p one) -> p one", one=1))
            f4w = wp.tile([D, 1], BF16, name="f4w")
            nc.sync.dma_start(f4w[:], fc4_w[:])
            f4b = wp.tile([P, 1], F32, name="f4b")
            nc.sync.dma_start(f4b[:], fc4_b[:])

            # ---------- phases B: 3 RGCN layers ----------
            for l in (1, 2, 3):
                basis, root, bias = rg_w[l]
                Bs = wp.tile([D, D], BF16, name=f"Bs_{l}")
                nc.sync.dma_start(Bs[:], basis[:])
                Rs = wp.tile([D, D], BF16, name=f"Rs_{l}")
                nc.sync.dma_start(Rs[:], root[:])
                bs = wp.tile([P, 1], F32, name=f"bs_{l}")
                nc.sync.dma_start(bs[:], bias.rearrange("(p one) -> p one", one=1))
                xcur = x_p[l - 1]
                xTc = xT_own[l - 1]
                xTn = xT_own[l]
                vwl = vw_sb[l]
                from contextlib import ExitStack
                with ExitStack() as stack:
                    gp = stack.enter_context(
                        tc.tile_pool(name=f"l{l}_g", bufs=2))
                    sp = stack.enter_context(
                        tc.tile_pool(name=f"l{l}_sb", bufs=3))
                    og = stack.enter_context(
                        tc.tile_pool(name=f"l{l}_og", bufs=2))
                    op = stack.enter_context(
                        tc.tile_pool(name=f"l{l}_o", bufs=16))
                    pp = stack.enter_context(
                        tc.tile_pool(name=f"l{l}_ps", bufs=2, space="PSUM"))
                    pq = stack.enter_context(
                        tc.tile_pool(name=f"l{l}_p2", bufs=2, space="PSUM"))
                    if l < 3:
                        pt = stack.enter_context(
                            tc.tile_pool(name=f"l{l}_pt", bufs=2,
                                         space="PSUM"))
                    else:
                        pt = stack.enter_context(
                            tc.tile_pool(name=f"l{l}_ph", bufs=2,
                                         space="PSUM"))
                        py = stack.enter_context(
                            tc.tile_pool(name=f"l{l}_py", bufs=2,
                                         space="PSUM"))
                    ly_pieces = 0
                    for g, sbs in enumerate(groups):
                        gb = grp_base[g]
                        xg = gp.tile([P, max_grp_tiles * P], BF16, name="xg")
                        for c in range(N_CORES):
                            nt = call_tiles[g][c]
                            if nt == 0:
                                continue
                            cs = grp_cstart[g][c]
                            nidx = nt * P
                            nc.gpsimd.dma_gather(
                                xg[:, cs * P:(cs + nt) * P].rearrange(
                                    "p (t e) -> p t e", e=D),
                                xcur[c][:, :],
                                idx_sb[:, (gb + cs) * 8:(gb + cs + nt) * 8],
                                nidx, nidx, D, single_packet=False,
                                queue_num=c % 4)
                        sub0 = 0
                        for sub_n in SUBS[g]:
                            sub = sbs[sub0:sub0 + sub_n]
                            s_base = sub[0]
                            MS = MSUB * SB
                            xtbg = og.tile([P, MS], BF16, name="xtbg")
                            nc.sync.dma_start(
                                xtbg[:, :sub_n * SB],
                                xTc[:, s_base * SB:(s_base + sub_n) * SB])
                            outg = og.tile([P, MS], BF16, name="outg")
                            if l < 3:
                                rowg = og.tile([P, MS], BF16, name="rowg")
                            is_var = l == 3 and s_base < NSB_VAR
                            if is_var:
                                gnv = sum(1 for s in sub if s < NSB_VAR)
                                xtls = []
                                for hl in range(3):
                                    xtl = og.tile([P, MS], BF16,
                                                  name=f"xtl{hl}")
                                    nc.sync.dma_start(
                                        xtl[:, :gnv * SB],
                                        xT_own[hl][:, s_base * SB:
                                                    (s_base + gnv) * SB])
                                    xtls.append(xtl)
                                ybuf = og.tile([P, 2 * MSUB], F32,
                                               name="ybuf")
                            for si, s in enumerate(sub):
                                so = (s - sbs[0])
                                cols = []
                                for c in range(N_CORES):
                                    if tpb2[s, c] == 0:
                                        continue
                                    c0 = grp_cstart[g][c] + sum(
                                        int(tpb2[s2, c]) for s2 in sbs[:so])
                                    for t in range(int(tpb2[s, c])):
                                        cols.append(c0 + t)
                                st = pp.tile([P, SB], F32, name="st",
                                             space="PSUM")
                                for ti, col in enumerate(cols):
                                    gcol = gb + col
                                    o = op.tile([P, SB], BF16, name="o")
                                    nc.vector.tensor_scalar(
                                        o[:], iota_t[:],
                                        dl_sb[:, gcol:gcol + 1],
                                        vwl[:, gcol:gcol + 1],
                                        op0=mybir.AluOpType.is_equal,
                                        op1=mybir.AluOpType.mult)
                                    nc.tensor.matmul(
                                        st[:],
                                        lhsT=xg[:, col * P:(col + 1) * P],
                                        rhs=o[:],
                                        start=(ti == 0),
                                        stop=(ti == len(cols) - 1))
                                p2 = pq.tile([P, SB], F32, name="p2",
                                             space="PSUM")
                                xtb = xtbg[:, si * SB:(si + 1) * SB]
                                if cols:
                                    sn = sp.tile([P, SB], BF16, name="sn")
                                    nc.scalar.activation(
                                        sn[:], st[:],
                                        mybir.ActivationFunctionType.Copy)
                                    nc.tensor.matmul(p2[:], lhsT=Bs[:],
                                                     rhs=sn[:],
                                                     start=True, stop=False)
                                    nc.tensor.matmul(
                                        p2[:], lhsT=Rs[:], rhs=xtb,
                                        start=False, stop=True)
                                else:
                                    nc.tensor.matmul(
                                        p2[:], lhsT=Rs[:], rhs=xtb,
                                        start=True, stop=True)
                                outs = outg[:, si * SB:(si + 1) * SB]
                                nc.scalar.activation(
                                    outs, p2[:],
                                    mybir.ActivationFunctionType.Relu,
                                    bias=bs[:, :1])
                                if l < 3:
                                    for h in range(2):
                                        c0 = si * SB + h * P
                                        tp2 = pt.tile([P, P], BF16,
                                                      name="tp2",
                                                      space="PSUM")
                                        nc.tensor.transpose(
                                            tp2[:], outg[:, c0:c0 + P],
                                            ident[:])
                                        nc.scalar.activation(
                                            rowg[:, c0:c0 + P], tp2[:],
                                            mybir.ActivationFunctionType.Copy)
                                if is_var and s < NSB_VAR:
                                    for h in range(2):
                                        cc0 = si * SB + h * P
                                        hp = pt.tile([P, P], F32, name="hp",
                                                     space="PSUM")
                                        for hl in range(3):
                                            nc.tensor.matmul(
                                                hp[:], lhsT=fc1c[hl][:],
                                                rhs=xtls[hl][:, cc0:cc0 + P],
                                                start=(hl == 0), stop=False)
                                        nc.tensor.matmul(
                                            hp[:], lhsT=fc1c[3][:],
                                            rhs=outg[:, cc0:cc0 + P],
                                            start=False, stop=True)
                                        hr = sp.tile([P, P], BF16, name="hr")
                                        nc.scalar.activation(
                                            hr[:], hp[:],
                                            mybir.ActivationFunctionType.Relu,
                                            bias=fb1[:, :1])
                                        yp = py.tile([P, 1], F32, name="yp",
                                                     space="PSUM")
                                        nc.tensor.matmul(yp[:], lhsT=hr[:],
                                                         rhs=f4w[:],
                                                         start=True,
                                                         stop=True)
                                        nc.vector.tensor_scalar(
                                            ybuf[:, si * 2 + h:si * 2 + h + 1],
                                            yp[:], f4b[:, :1], None,
                                            op0=mybir.AluOpType.add)
                            nc.sync.dma_start(
                                xTn[:, s_base * SB:(s_base + sub_n) * SB],
                                outg[:, :sub_n * SB])
                            if l < 3:
                                r0 = s_base * SB
                                nc.sync.dma_start(
                                    ag_in[l - 1][r0:r0 + sub_n * SB, :]
                                    .rearrange("(t p) d -> p t d", p=P),
                                    rowg[:, :sub_n * SB].rearrange(
                                        "p (t d) -> p t d", d=D))
                            if is_var:
                                nc.sync.dma_start(
                                    y_out[s_base * SB:(s_base + gnv) * SB]
                                    .rearrange("(g p) -> p g", p=P),
                                    ybuf[:, :gnv * 2])
                            sub0 += sub_n
                            if l < 3:
                                rows_done = (s_base + sub_n) * SB
                                while (ly_pieces < NPIECE
                                       and rows_done
                                       >= (ly_pieces + 1) * PIECE_ROWS):
                                    pz = ly_pieces
                                    nc.gpsimd.collective_compute(
                                        "AllGather", mybir.AluOpType.bypass,
                                        replica_groups=rgroups,
                                        ins=[ag_in[l - 1][
                                            pz * PIECE_ROWS:
                                            (pz + 1) * PIECE_ROWS, :]],
                                        outs=[x_p[l][pz][:]])
                                    ly_pieces += 1


    nc.compile()
    return nc


_CACHE = {}


def kernel(**inputs) -> np.ndarray:
    from concourse import bass_utils
    in_maps, tpb2, groups, call_tiles, T2, var_slot, key = _preprocess(inputs)
    if key not in _CACHE:
        _CACHE[key] = _build_program(tpb2, groups, call_tiles, T2)
    nc = _CACHE[key]
    res = bass_utils.run_bass_kernel_spmd(
        nc, in_maps, core_ids=list(range(N_CORES)))
    y = np.empty(N_VAR, np.float32)
    for c in range(N_CORES):
        ys = res.results[c]["y_out"]
        vids = np.arange(c * NV_CORE, (c + 1) * NV_CORE)
        y[vids] = ys[var_slot[vids]]
    return y.astype(np.float32)


# revision 35
# speedup vs baseline: 1.0720x; 1.0720x over previous
"""Trainium2 Bass kernel for nn_Net_69114613727316 (RGCN message passing).

Self-contained: kernel(**inputs) -> np.ndarray [100000] float32.

Math (exploiting num_bases=1): w[r] = att[r,0] * basis, so
    agg_d = ((sum_e v_e * x[src_e]) @ basis),  v_e = att_l[etype_e]/max(cnt_d,1)
    out = relu(agg + x @ root + bias)
Per-edge GEMMs collapse into a weighted scatter-add (one-hot matmuls into
PSUM per 256-dst "superband" window) plus one dense GEMM per window.

Sharding: node space is permuted so core c owns rows
[c*25088, (c+1)*25088) = [12544 var slots | 12544 con slots].  Each
layer's full x (bf16) is replicated via piece-wise AllGather so any core
can gather arbitrary src rows; each core computes only its own rows.

v6 (from v5):
  * degree-balanced node->slot packing per core: per-(superband, chunk)
    edge counts equalized so every (sb, chunk) needs exactly 1 128-edge
    tile; zero-in-degree nodes concentrated in dedicated superbands
    whose scatter is skipped entirely
  * stripe-mined gathers: one dma_gather per (24..26-superband stripe,
    src chunk) -> 32 calls/layer instead of 72
  * piece-wise AllGather (4 pieces per layer, launched as each stripe's
    rows finish) overlapping collective latency under compute
  * per-superband small stores (outg/rowg/xT) to cut SBUF pressure
  * phase-A input MLPs processed in 256-column pairs
"""
import numpy as np
import ml_dtypes

import concourse.bass as bass
import concourse.bacc as bacc
import concourse.tile as tile
import concourse.mybir as mybir
from concourse import library_config

F32 = mybir.dt.float32
BF16 = mybir.dt.bfloat16
I16 = mybir.dt.int16
D = 128
P = 128
SB = 256                      # superband width (dst window)

N_VAR = 100000
N_CON = 100000
N_EDGES = 640000
N_CORES = 8

NV_CORE = N_VAR // N_CORES          # 12500
NC_CORE = N_CON // N_CORES          # 12500
NV_SLOT = ((NV_CORE + P - 1) // P) * P   # 12544
NC_SLOT = ((NC_CORE + P - 1) // P) * P   # 12544
PER_CORE = NV_SLOT + NC_SLOT        # 25088
N_PAD = N_CORES * PER_CORE          # 200704
NSB = PER_CORE // SB                # 98 superbands
NSB_VAR = NV_SLOT // SB             # 49 (sbs 0..48 var, 49..97 con)

GS = [20, 20, 20, 20, 18]           # gather stripe sizes (superbands)
SUBS = [[10, 10], [10, 10], [10, 10], [10, 10], [9, 9]]  # og sub-blocks
MSUB = max(max(s) for s in SUBS)    # max sub width (superbands)
# x replication pieces: (slot row0, rows per core). Each piece is one
# AllGather into a [8*rows, D] tensor; gather chunks are 25088-row slices
# of piece tensors (25088/rows cores per slice), 8 chunks total.
PIECES = [(0, 12544), (12544, 6272), (18816, 3136), (21952, 3136)]
_CH = []
for _pz, (_r0, _rp) in enumerate(PIECES):
    _mc = PER_CORE // _rp
    for _j in range(N_CORES // _mc):
        _CH.append((_pz, _j, _r0, _rp, _mc))
assert len(_CH) == N_CORES
NCHUNK = N_CORES

BF = ml_dtypes.bfloat16


def _bf(a):
    return np.asarray(a, np.float32).astype(BF)


def _pack_nodes(deg_chunk, nsb_zero=1):
    """Assign node-local-ids (0..n-1) to slots within 49 superbands.

    deg_chunk: [n, 8] per-chunk in-degree. Returns slot_of_node [n].
    Last `nsb_zero` superbands get only zero-degree nodes (if enough),
    remaining nodes snake-dealt by total degree into the other sbs.
    """
    n = deg_chunk.shape[0]
    nsb = 49
    tot = deg_chunk.sum(axis=1)
    order = np.argsort(-tot, kind="stable")
    zero_ids = order[tot[order] == 0]
    nz_ids = order[tot[order] > 0]
    slot = np.full(n, -1, np.int64)
    # fill zero sbs from the back with zero-degree nodes
    zcap = nsb_zero * SB
    if len(zero_ids) < zcap:
        # not enough true-zero nodes: pull lowest-degree nodes too
        extra = nz_ids[len(nz_ids) - (zcap - len(zero_ids)):]
        nz_ids = nz_ids[:len(nz_ids) - (zcap - len(zero_ids))]
        zfill = np.concatenate([zero_ids, extra])
    else:
        zfill = zero_ids[:zcap]
        nz_ids = np.concatenate([nz_ids, zero_ids[zcap:]])
    slot[zfill] = (nsb - nsb_zero) * SB + np.arange(zcap)
    # greedy chunk-aware packing of nonzero nodes (degree-desc): place each
    # node in the sb minimizing the resulting max per-chunk count
    nact = nsb - nsb_zero
    k = len(nz_ids)
    cnt = np.zeros((nact, N_CORES), np.int64)
    used = np.zeros(nact, np.int64)
    sb_of = np.zeros(k, np.int64)
    degs = deg_chunk[nz_ids]
    for i in range(k):
        dn = degs[i]
        score = (cnt + dn).max(axis=1) * 1024 + used
        score[used >= SB] = 1 << 60
        s = int(np.argmin(score))
        sb_of[i] = s
        cnt[s] += dn
        used[s] += 1
    # rank within sb = order of assignment
    rank = np.zeros(k, np.int64)
    srt = np.argsort(sb_of, kind="stable")
    sb_sorted = sb_of[srt]
    starts = np.searchsorted(sb_sorted, np.arange(nact))
    ends = np.searchsorted(sb_sorted, np.arange(nact), side="right")
    for s in range(nact):
        sel = srt[starts[s]:ends[s]]
        rank[sel] = np.arange(ends[s] - starts[s])
    assert (rank < SB).all(), "superband overflow in packing"
    slot[nz_ids] = sb_of * SB + rank
    assert (slot >= 0).all()
    return slot


def _preprocess(inputs):
    vf = np.ascontiguousarray(np.asarray(inputs["var_node_features"], np.float32))
    cf = np.ascontiguousarray(np.asarray(inputs["con_node_features"], np.float32))
    ei = np.asarray(inputs["edge_index"])
    et = np.asarray(inputs["edge_types"]).astype(np.int64)
    assert (np.asarray(inputs["assoc_var"]) == np.arange(N_VAR)).all()
    assert (np.asarray(inputs["assoc_con"]) == N_VAR + np.arange(N_CON)).all()

    src = ei[0].astype(np.int64)
    dst = ei[1].astype(np.int64)

    # node -> (core, kind, local id): var v -> core v//NV_CORE, local v%NV_CORE
    node = np.arange(N_VAR + N_CON)
    isv_n = node < N_VAR
    core_n = np.where(isv_n, node // NV_CORE, (node - N_VAR) // NC_CORE)
    loc_n = np.where(isv_n, node % NV_CORE, (node - N_VAR) % NC_CORE)

    # per-node per-src-chunk in-degree (chunk = src core after permute —
    # but chunk of an edge depends on the SRC node's core, known already)
    src_core = np.where(src < N_VAR, src // NV_CORE, (src - N_VAR) // NC_CORE)
    # in-degree per (dst node, src core)
    degc = np.zeros((N_VAR + N_CON, N_CORES), np.int64)
    np.add.at(degc, (dst, src_core), 1)

    # pack per core: var + con separately; iterate with src-piece-aware
    # degree vectors (gather chunk = slot-range piece) and keep the best
    def pack_all(dc):
        slot = np.zeros(N_VAR + N_CON, np.int64)
        for c in range(N_CORES):
            vids = np.arange(c * NV_CORE, (c + 1) * NV_CORE)
            slot[vids] = _pack_nodes(dc[vids], nsb_zero=1)
            cids = N_VAR + np.arange(c * NC_CORE, (c + 1) * NC_CORE)
            slot[cids] = NV_SLOT + _pack_nodes(dc[cids], nsb_zero=1)
        return slot

    _row0s = np.array([r0 for r0, _ in PIECES] + [PER_CORE])
    _base = np.cumsum([0] + [N_CORES // (PER_CORE // rp) for _, rp in PIECES])

    def chunk_of(pm_src):
        sl = pm_src % PER_CORE
        c = pm_src // PER_CORE
        pz = np.searchsorted(_row0s, sl, side="right") - 1
        rp = np.array([rp for _, rp in PIECES])[pz]
        mc = PER_CORE // rp
        ch = _base[pz] + c // mc
        ls = (c % mc) * rp + (sl - _row0s[pz])
        return ch, ls

    def t2_of(slot):
        pm = core_n * PER_CORE + slot
        pc, _ = chunk_of(pm[src])
        sb_ = (pm[dst] % PER_CORE) // SB
        sg = ((pm[dst] // PER_CORE) * NSB + sb_) * NCHUNK + pc
        cn = np.bincount(sg, minlength=N_CORES * NSB * NCHUNK).reshape(
            N_CORES, NSB, NCHUNK)
        return int(np.ceil(cn.max(axis=0) / P).sum())

    slot_of_node = pack_all(degc)
    best = (t2_of(slot_of_node), slot_of_node)
    cur = slot_of_node
    for _ in range(2):
        pm = core_n * PER_CORE + cur
        piece_lbl, _ = chunk_of(pm[src])
        degc2 = np.zeros((N_VAR + N_CON, NCHUNK), np.int64)
        np.add.at(degc2, (dst, piece_lbl), 1)
        cur = pack_all(degc2)
        t2v = t2_of(cur)
        if t2v < best[0]:
            best = (t2v, cur)
    slot_of_node = best[1]

    perm = core_n * PER_CORE + slot_of_node    # node -> padded row
    psrc = perm[src]
    pdst = perm[dst]
    cnt = np.bincount(pdst, minlength=N_PAD).astype(np.float32)
    invc = 1.0 / np.maximum(cnt, 1.0)
    atts = [np.asarray(inputs[f"att{l}"], np.float32)[:, 0] for l in (1, 2, 3)]

    core = pdst // PER_CORE
    sb = (pdst % PER_CORE) // SB          # superband 0..97
    dl2 = (pdst % SB).astype(np.float32)  # dst offset in window, 0..255
    chunk, lsrc64 = chunk_of(psrc)
    lsrc = lsrc64.astype(np.int16)

    # tiles per (superband, chunk): shared across cores (max)
    seg = (core * NSB + sb) * N_CORES + chunk
    counts = np.bincount(seg, minlength=N_CORES * NSB * N_CORES).reshape(
        N_CORES, NSB, N_CORES)
    tpb2 = np.ceil(counts.max(axis=0) / P).astype(np.int64)  # [NSB, CHUNKS]

    # canonical tile-column order: stripe G -> chunk c -> sb in G -> tiles
    assert sum(GS) == NSB
    groups = []
    s0 = 0
    for gsz in GS:
        groups.append(list(range(s0, s0 + gsz)))
        s0 += gsz
    colmap = {}
    ncol = 0
    for g, sbs in enumerate(groups):
        for c in range(N_CORES):
            for s in sbs:
                colmap[(s, c)] = ncol
                ncol += int(tpb2[s, c])
    T2 = ncol

    # rank of each edge within its (core, sb, chunk) run
    order = np.argsort(seg, kind="stable")
    seg_s = seg[order]
    run_starts = np.concatenate(
        [[0], np.cumsum(np.bincount(seg_s, minlength=seg.max() + 1))[:-1]])
    rank = np.arange(len(order)) - run_starts[seg_s]
    core_s = core[order]
    col0 = np.array([colmap[(s, c)] for s, c in
                     zip(sb[order].tolist(), chunk[order].tolist())])
    tcol = col0 + rank // P
    prow = rank % P

    lsrc_arr = np.zeros((N_CORES, P, T2), np.int16)
    dl_arr = np.zeros((N_CORES, P, T2), np.float32)
    vw_arr = np.zeros((3, N_CORES, P, T2), np.float32)
    lsrc_arr[core_s, prow, tcol] = lsrc[order]
    dl_arr[core_s, prow, tcol] = dl2[order]
    for li in range(3):
        vw_arr[li, core_s, prow, tcol] = atts[li][et[order]] * invc[pdst[order]]

    # int16 idxs in dma_gather layout: for linear j in a call, value at
    # partition 16*r + j%16 (replicated r=0..7), column tcol*8 + prow//16.
    idx16 = np.zeros((N_CORES, P, T2 * 8), np.int16)
    tc_all = np.repeat(np.arange(T2), P)
    pr_all = np.tile(np.arange(P), T2)
    cols = tc_all * 8 + pr_all // 16
    rows_ = pr_all % 16
    for cidx in range(N_CORES):
        vals = lsrc_arr[cidx, pr_all, tc_all]
        for r in range(8):
            idx16[cidx, 16 * r + rows_, cols] = vals

    # per-(stripe,chunk) call shapes
    call_tiles = []
    for g, sbs in enumerate(groups):
        call_tiles.append(
            [int(sum(int(tpb2[s, c]) for s in sbs)) for c in range(N_CORES)])

    iota256 = np.broadcast_to(
        np.arange(SB, dtype=np.float32), (P, SB)).astype(BF)
    ident = np.eye(P, dtype=np.float32).astype(BF)

    # per-core permuted features (staged per slot)
    in_maps = []
    for c in range(N_CORES):
        vfeat = np.zeros((NV_SLOT, 2), np.float32)
        vids = np.arange(c * NV_CORE, (c + 1) * NV_CORE)
        vfeat[slot_of_node[vids]] = vf[vids]
        cfeat = np.zeros((NC_SLOT, 2), np.float32)
        cids = np.arange(c * NC_CORE, (c + 1) * NC_CORE)
        cfeat[slot_of_node[N_VAR + cids] - NV_SLOT] = cf[cids]
        m = {
            "vfeatT": _bf(vfeat.T),
            "cfeatT": _bf(cfeat.T),
            "idx16": idx16[c],
            "dl2": dl_arr[c],
            "vw1": vw_arr[0, c],
            "vw2": vw_arr[1, c],
            "vw3": vw_arr[2, c],
            "iota256": iota256,
            "ident": ident,
            "fc1_w": _bf(inputs["fc1_w"]),
            "fc1_b": np.asarray(inputs["fc1_b"], np.float32),
            "fc4_w": _bf(inputs["fc4_w"]),
            "fc4_b": np.broadcast_to(
                np.asarray(inputs["fc4_b"], np.float32).reshape(1, 1),
                (P, 1)).copy(),
        }
        for t in ("var", "con"):
            m[f"{t}_w1"] = _bf(inputs[f"{t}_w1"])
            m[f"{t}_b1"] = np.asarray(inputs[f"{t}_b1"], np.float32)
            m[f"{t}_w2"] = _bf(inputs[f"{t}_w2"])
            m[f"{t}_b2"] = np.asarray(inputs[f"{t}_b2"], np.float32)
        for l in (1, 2, 3):
            m[f"basis{l}"] = _bf(np.asarray(inputs[f"basis{l}"],
                                            np.float32).reshape(D, D))
            m[f"root{l}"] = _bf(inputs[f"root{l}"])
            m[f"bias{l}"] = np.asarray(inputs[f"bias{l}"], np.float32)
        in_maps.append(m)
    # var slot -> global var id map for output assembly
    var_slot = slot_of_node[:N_VAR]    # slot within core var block
    key = tuple(int(t) for t in tpb2.reshape(-1))
    return in_maps, tpb2, groups, call_tiles, T2, var_slot, key


def _build_program(tpb2, groups, call_tiles, T2):
    nc = bacc.Bacc("TRN2", target_bir_lowering=False, debug=False,
                   num_devices=N_CORES, num_swdge_queues=4)

    def inp(name, shape, dtype=BF16):
        return nc.dram_tensor(name, shape, dtype, kind="ExternalInput")

    vfeatT = inp("vfeatT", [2, NV_SLOT])
    cfeatT = inp("cfeatT", [2, NC_SLOT])
    idx16 = inp("idx16", [P, T2 * 8], I16)
    dl2 = inp("dl2", [P, T2], F32)
    vws = {l: inp(f"vw{l}", [P, T2], F32) for l in (1, 2, 3)}
    iota256 = inp("iota256", [P, SB])
    identt = inp("ident", [P, P])
    mlp_w = {}
    for t in ("var", "con"):
        mlp_w[t] = (inp(f"{t}_w1", [2, D]), inp(f"{t}_b1", [D], F32),
                    inp(f"{t}_w2", [D, D]), inp(f"{t}_b2", [D], F32))
    rg_w = {l: (inp(f"basis{l}", [D, D]), inp(f"root{l}", [D, D]),
                inp(f"bias{l}", [D], F32)) for l in (1, 2, 3)}
    fc1_w = inp("fc1_w", [4 * D, D])
    fc1_b = inp("fc1_b", [D], F32)
    fc4_w = inp("fc4_w", [D, 1])
    fc4_b = inp("fc4_b", [P, 1], F32)
    y_out = nc.dram_tensor("y_out", [NV_SLOT], F32, kind="ExternalOutput")

    x_p = [[nc.dram_tensor(f"x{i}_p{p}", [N_CORES * rp, D], BF16,
                           kind="Internal", addr_space="Shared")
            for p, (_, rp) in enumerate(PIECES)] for i in range(3)]
    ag_in = [nc.dram_tensor(f"ag_in{i}", [PER_CORE, D], BF16, kind="Internal")
             for i in range(3)]
    xT_own = [nc.dram_tensor(f"xT{i}_own", [D, PER_CORE], BF16,
                             kind="Internal") for i in range(4)]

    rgroups = [list(range(N_CORES))]
    grp_cstart = []
    grp_base = []
    base = 0
    for g in range(len(groups)):
        cst = []
        off = 0
        for c in range(N_CORES):
            cst.append(off)
            off += call_tiles[g][c]
        grp_cstart.append(cst)
        grp_base.append(base)
        base += off
    max_grp_tiles = max(sum(ct) for ct in call_tiles)

    with tile.TileContext(nc) as tc:
        with tc.tile_pool(name="wp", bufs=1) as wp:
            nc.gpsimd.load_library(library_config.mlp)
            iota_t = wp.tile([P, SB], BF16, name="iota_t")
            nc.sync.dma_start(iota_t[:], iota256[:])
            ident = wp.tile([P, P], BF16, name="ident")
            nc.sync.dma_start(ident[:], identt[:])

            idx_sb = wp.tile([P, T2 * 8], I16, name="idx_sb")
            nc.sync.dma_start(idx_sb[:], idx16[:])
            dl_sb = wp.tile([P, T2], F32, name="dl_sb")
            nc.sync.dma_start(dl_sb[:], dl2[:])
            vw_sb = {}
            for l in (1, 2, 3):
                vw_sb[l] = wp.tile([P, T2], F32, name=f"vw_sb{l}")
                nc.sync.dma_start(vw_sb[l][:], vws[l][:])

            # ---------- phase A: input MLPs -> x0 (pairs of bands) ----------
            with tc.tile_pool(name="pa_f", bufs=1) as fp, \
                 tc.tile_pool(name="pa_sb", bufs=3) as sp, \
                 tc.tile_pool(name="pa_o", bufs=4) as op_a, \
                 tc.tile_pool(name="pa_pst", bufs=2, space="PSUM") as pp_t, \
                 tc.tile_pool(name="pa_ps", bufs=2, space="PSUM") as pp:
                pa_pieces = 0
                for ttype, featT, slot0, nslot in (
                    ("var", vfeatT, 0, NV_SLOT),
                    ("con", cfeatT, NV_SLOT, NC_SLOT),
                ):
                    w1, b1, w2, b2 = mlp_w[ttype]
                    w1s = wp.tile([2, D], BF16, name=f"w1s_{ttype}")
                    nc.sync.dma_start(w1s[:], w1[:])
                    b1s = wp.tile([P, 1], F32, name=f"b1s_{ttype}")
                    nc.sync.dma_start(b1s[:], b1.rearrange("(p one) -> p one", one=1))
                    w2s = wp.tile([D, D], BF16, name=f"w2s_{ttype}")
                    nc.sync.dma_start(w2s[:], w2[:])
                    b2s = wp.tile([P, 1], F32, name=f"b2s_{ttype}")
                    nc.sync.dma_start(b2s[:], b2.rearrange("(p one) -> p one", one=1))
                    ftT = fp.tile([2, nslot], BF16, name=f"ftT_{ttype}")
                    nc.sync.dma_start(ftT[:], featT[:])
                    npairs = nslot // SB
                    for j in range(npairs):
                        p1 = pp.tile([P, SB], F32, name="p1", space="PSUM")
                        nc.tensor.matmul(p1[:], lhsT=w1s[:],
                                         rhs=ftT[:, j * SB:(j + 1) * SB],
                                         start=True, stop=True)
                        h1 = sp.tile([P, SB], BF16, name="h1")
                        nc.scalar.activation(h1[:], p1[:],
                                             mybir.ActivationFunctionType.Relu,
                                             bias=b1s[:, :1])
                        p2 = pp.tile([P, SB], F32, name="p2", space="PSUM")
                        nc.tensor.matmul(p2[:], lhsT=w2s[:], rhs=h1[:],
                                         start=True, stop=True)
                        xTg = op_a.tile([P, SB], BF16, name="xTg")
                        nc.vector.tensor_scalar(
                            xTg[:], p2[:],
                            b2s[:, :1], None, op0=mybir.AluOpType.add)
                        rowg = op_a.tile([P, SB], BF16, name="rowg")
                        for h in range(2):
                            tp = pp_t.tile([P, P], BF16, name="tp",
                                           space="PSUM")
                            nc.tensor.transpose(
                                tp[:], xTg[:, h * P:(h + 1) * P], ident[:])
                            nc.scalar.activation(
                                rowg[:, h * P:(h + 1) * P], tp[:],
                                mybir.ActivationFunctionType.Copy)
                        col0 = slot0 + j * SB
                        nc.sync.dma_start(
                            xT_own[0][:, col0:col0 + SB], xTg[:])
                        nc.sync.dma_start(
                            ag_in[0][col0:col0 + SB, :]
                            .rearrange("(t p) d -> p t d", p=P),
                            rowg[:].rearrange("p (t d) -> p t d", d=D))
                        while (pa_pieces < len(PIECES)
                               and col0 + SB >= (PIECES[pa_pieces][0]
                                                 + PIECES[pa_pieces][1])):
                            pz = pa_pieces
                            r0, rp = PIECES[pz]
                            nc.gpsimd.collective_compute(
                                "AllGather", mybir.AluOpType.bypass,
                                replica_groups=rgroups,
                                ins=[ag_in[0][r0:r0 + rp, :]],
                                outs=[x_p[0][pz][:]])
                            pa_pieces += 1

            # head-MLP weights (consumed inside layer 3)
            fc1c = []
            for hl in range(4):
                t = wp.tile([D, D], BF16, name=f"fc1c{hl}")
                nc.sync.dma_start(t[:], fc1_w[hl * D:(hl + 1) * D, :])
                fc1c.append(t)
            fb1 = wp.tile([P, 1], F32, name="fb1")
            nc.sync.dma_start(fb1[:], fc1_b.rearrange("(p one) -> p one", one=1))
            f4w = wp.tile([D, 1], BF16, name="f4w")
            nc.sync.dma_start(f4w[:], fc4_w[:])
            f4b = wp.tile([P, 1], F32, name="f4b")
            nc.sync.dma_start(f4b[:], fc4_b[:])

            # ---------- phases B: 3 RGCN layers ----------
            for l in (1, 2, 3):
                basis, root, bias = rg_w[l]
                Bs = wp.tile([D, D], BF16, name=f"Bs_{l}")
                nc.sync.dma_start(Bs[:], basis[:])
                Rs = wp.tile([D, D], BF16, name=f"Rs_{l}")
                nc.sync.dma_start(Rs[:], root[:])
                bs = wp.tile([P, 1], F32, name=f"bs_{l}")
                nc.sync.dma_start(bs[:], bias.rearrange("(p one) -> p one", one=1))
                xcur = x_p[l - 1]
                xTc = xT_own[l - 1]
                xTn = xT_own[l]
                vwl = vw_sb[l]
                from contextlib import ExitStack
                with ExitStack() as stack:
                    gp = stack.enter_context(
                        tc.tile_pool(name=f"l{l}_g", bufs=2))
                    sp = stack.enter_context(
                        tc.tile_pool(name=f"l{l}_sb", bufs=3))
                    og = stack.enter_context(
                        tc.tile_pool(name=f"l{l}_og", bufs=2))
                    op = stack.enter_context(
                        tc.tile_pool(name=f"l{l}_o", bufs=16))
                    pp = stack.enter_context(
                        tc.tile_pool(name=f"l{l}_ps", bufs=2, space="PSUM"))
                    pq = stack.enter_context(
                        tc.tile_pool(name=f"l{l}_p2", bufs=2, space="PSUM"))
                    if l < 3:
                        pt = stack.enter_context(
                            tc.tile_pool(name=f"l{l}_pt", bufs=2,
                                         space="PSUM"))
                    else:
                        pt = stack.enter_context(
                            tc.tile_pool(name=f"l{l}_ph", bufs=2,
                                         space="PSUM"))
                        py = stack.enter_context(
                            tc.tile_pool(name=f"l{l}_py", bufs=2,
                                         space="PSUM"))
                    ly_pieces = 0
                    for g, sbs in enumerate(groups):
                        gb = grp_base[g]
                        xg = gp.tile([P, max_grp_tiles * P], BF16, name="xg")
                        for c in range(N_CORES):
                            nt = call_tiles[g][c]
                            if nt == 0:
                                continue
                            cs = grp_cstart[g][c]
                            nidx = nt * P
                            pz, jj = _CH[c][0], _CH[c][1]
                            nc.gpsimd.dma_gather(
                                xg[:, cs * P:(cs + nt) * P].rearrange(
                                    "p (t e) -> p t e", e=D),
                                xcur[pz][jj * PER_CORE:(jj + 1) * PER_CORE, :],
                                idx_sb[:, (gb + cs) * 8:(gb + cs + nt) * 8],
                                nidx, nidx, D, single_packet=False,
                                queue_num=c % 4)
                        sub0 = 0
                        for sub_n in SUBS[g]:
                            sub = sbs[sub0:sub0 + sub_n]
                            s_base = sub[0]
                            MS = MSUB * SB
                            xtbg = og.tile([P, MS], BF16, name="xtbg")
                            nc.sync.dma_start(
                                xtbg[:, :sub_n * SB],
                                xTc[:, s_base * SB:(s_base + sub_n) * SB])
                            outg = og.tile([P, MS], BF16, name="outg")
                            if l < 3:
                                rowg = og.tile([P, MS], BF16, name="rowg")
                            is_var = l == 3 and s_base < NSB_VAR
                            if is_var:
                                gnv = sum(1 for s in sub if s < NSB_VAR)
                                xtls = []
                                for hl in range(3):
                                    xtl = og.tile([P, MS], BF16,
                                                  name=f"xtl{hl}")
                                    nc.sync.dma_start(
                                        xtl[:, :gnv * SB],
                                        xT_own[hl][:, s_base * SB:
                                                    (s_base + gnv) * SB])
                                    xtls.append(xtl)
                                ybuf = og.tile([P, 2 * MSUB], F32,
                                               name="ybuf")
                            for si, s in enumerate(sub):
                                so = (s - sbs[0])
                                cols = []
                                for c in range(N_CORES):
                                    if tpb2[s, c] == 0:
                                        continue
                                    c0 = grp_cstart[g][c] + sum(
                                        int(tpb2[s2, c]) for s2 in sbs[:so])
                                    for t in range(int(tpb2[s, c])):
                                        cols.append(c0 + t)
                                st = pp.tile([P, SB], F32, name="st",
                                             space="PSUM")
                                for ti, col in enumerate(cols):
                                    gcol = gb + col
                                    o = op.tile([P, SB], BF16, name="o")
                                    nc.vector.tensor_scalar(
                                        o[:], iota_t[:],
                                        dl_sb[:, gcol:gcol + 1],
                                        vwl[:, gcol:gcol + 1],
                                        op0=mybir.AluOpType.is_equal,
                                        op1=mybir.AluOpType.mult)
                                    nc.tensor.matmul(
                                        st[:],
                                        lhsT=xg[:, col * P:(col + 1) * P],
                                        rhs=o[:],
                                        start=(ti == 0),
                                        stop=(ti == len(cols) - 1))
                                p2 = pq.tile([P, SB], F32, name="p2",
                                             space="PSUM")
                                xtb = xtbg[:, si * SB:(si + 1) * SB]
                                if cols:
                                    sn = sp.tile([P, SB], BF16, name="sn")
                                    nc.scalar.activation(
                                        sn[:], st[:],
                                        mybir.ActivationFunctionType.Copy)
                                    nc.tensor.matmul(p2[:], lhsT=Bs[:],
                                                     rhs=sn[:],
                                                     start=True, stop=False)
                                    nc.tensor.matmul(
                                        p2[:], lhsT=Rs[:], rhs=xtb,
                                        start=False, stop=True)
                                else:
                                    nc.tensor.matmul(
                                        p2[:], lhsT=Rs[:], rhs=xtb,
                                        start=True, stop=True)
                                outs = outg[:, si * SB:(si + 1) * SB]
                                nc.scalar.activation(
                                    outs, p2[:],
                                    mybir.ActivationFunctionType.Relu,
                                    bias=bs[:, :1])
                                if l < 3:
                                    for h in range(2):
                                        c0 = si * SB + h * P
                                        tp2 = pt.tile([P, P], BF16,
                                                      name="tp2",
                                                      space="PSUM")
                                        nc.tensor.transpose(
                                            tp2[:], outg[:, c0:c0 + P],
                                            ident[:])
                                        nc.scalar.activation(
                                            rowg[:, c0:c0 + P], tp2[:],
                                            mybir.ActivationFunctionType.Copy)
                                if is_var and s < NSB_VAR:
                                    for h in range(2):
                                        cc0 = si * SB + h * P
                                        hp = pt.tile([P, P], F32, name="hp",
                                                     space="PSUM")
                                        for hl in range(3):
                                            nc.tensor.matmul(
                                                hp[:], lhsT=fc1c[hl][:],
                                                rhs=xtls[hl][:, cc0:cc0 + P],
                                                start=(hl == 0), stop=False)
                                        nc.tensor.matmul(
                                            hp[:], lhsT=fc1c[3][:],
                                            rhs=outg[:, cc0:cc0 + P],
                                            start=False, stop=True)
                                        hr = sp.tile([P, P], BF16, name="hr")
                                        nc.scalar.activation(
                                            hr[:], hp[:],
                                            mybir.ActivationFunctionType.Relu,
                                            bias=fb1[:, :1])
                                        yp = py.tile([P, 1], F32, name="yp",
                                                     space="PSUM")
                                        nc.tensor.matmul(yp[:], lhsT=hr[:],
                                                         rhs=f4w[:],
                                                         start=True,
                                                         stop=True)
                                        nc.vector.tensor_scalar(
                                            ybuf[:, si * 2 + h:si * 2 + h + 1],
                                            yp[:], f4b[:, :1], None,
                                            op0=mybir.AluOpType.add)
                            nc.sync.dma_start(
                                xTn[:, s_base * SB:(s_base + sub_n) * SB],
                                outg[:, :sub_n * SB])
                            if l < 3:
                                r0 = s_base * SB
                                nc.sync.dma_start(
                                    ag_in[l - 1][r0:r0 + sub_n * SB, :]
                                    .rearrange("(t p) d -> p t d", p=P),
                                    rowg[:, :sub_n * SB].rearrange(
                                        "p (t d) -> p t d", d=D))
                            if is_var:
                                nc.sync.dma_start(
                                    y_out[s_base * SB:(s_base + gnv) * SB]
                                    .rearrange("(g p) -> p g", p=P),
                                    ybuf[:, :gnv * 2])
                            sub0 += sub_n
                            if l < 3:
                                rows_done = (s_base + sub_n) * SB
                                while (ly_pieces < len(PIECES)
                                       and rows_done
                                       >= (PIECES[ly_pieces][0]
                                           + PIECES[ly_pieces][1])):
                                    pz = ly_pieces
                                    r0, rp = PIECES[pz]
                                    nc.gpsimd.collective_compute(
                                        "AllGather", mybir.AluOpType.bypass,
                                        replica_groups=rgroups,
                                        ins=[ag_in[l - 1][r0:r0 + rp, :]],
                                        outs=[x_p[l][pz][:]])
                                    ly_pieces += 1


    nc.compile()
    return nc


_CACHE = {}


def kernel(**inputs) -> np.ndarray:
    from concourse import bass_utils
    in_maps, tpb2, groups, call_tiles, T2, var_slot, key = _preprocess(inputs)
    if key not in _CACHE:
        _CACHE[key] = _build_program(tpb2, groups, call_tiles, T2)
    nc = _CACHE[key]
    res = bass_utils.run_bass_kernel_spmd(
        nc, in_maps, core_ids=list(range(N_CORES)))
    y = np.empty(N_VAR, np.float32)
    for c in range(N_CORES):
        ys = res.results[c]["y_out"]
        vids = np.arange(c * NV_CORE, (c + 1) * NV_CORE)
        y[vids] = ys[var_slot[vids]]
    return y.astype(np.float32)


# revision 51
# speedup vs baseline: 1.1575x; 1.0797x over previous
"""Trainium2 Bass kernel for nn_Net_69114613727316 (RGCN message passing).

Self-contained: kernel(**inputs) -> np.ndarray [100000] float32.

Math (exploiting num_bases=1): w[r] = att[r,0] * basis, so
    agg_d = ((sum_e v_e * x[src_e]) @ basis),  v_e = att_l[etype_e]/max(cnt_d,1)
    out = relu(agg + x @ root + bias)
Per-edge GEMMs collapse into a weighted scatter-add (one-hot matmuls into
PSUM per 256-dst "superband" window) plus one dense GEMM per window.

Sharding: node space is permuted so core c owns rows
[c*25088, (c+1)*25088) = [12544 var slots | 12544 con slots].  Each
layer's full x (bf16) is replicated via piece-wise AllGather so any core
can gather arbitrary src rows; each core computes only its own rows.

v6 (from v5):
  * degree-balanced node->slot packing per core: per-(superband, chunk)
    edge counts equalized so every (sb, chunk) needs exactly 1 128-edge
    tile; zero-in-degree nodes concentrated in dedicated superbands
    whose scatter is skipped entirely
  * stripe-mined gathers: one dma_gather per (24..26-superband stripe,
    src chunk) -> 32 calls/layer instead of 72
  * piece-wise AllGather (4 pieces per layer, launched as each stripe's
    rows finish) overlapping collective latency under compute
  * per-superband small stores (outg/rowg/xT) to cut SBUF pressure
  * phase-A input MLPs processed in 256-column pairs
"""
import numpy as np
import ml_dtypes

import concourse.bass as bass
import concourse.bacc as bacc
import concourse.tile as tile
import concourse.mybir as mybir
from concourse import library_config

F32 = mybir.dt.float32
BF16 = mybir.dt.bfloat16
I16 = mybir.dt.int16
D = 128
P = 128
SB = 256                      # superband width (dst window)

N_VAR = 100000
N_CON = 100000
N_EDGES = 640000
N_CORES = 8

NV_CORE = N_VAR // N_CORES          # 12500
NC_CORE = N_CON // N_CORES          # 12500
NV_SLOT = ((NV_CORE + P - 1) // P) * P   # 12544
NC_SLOT = ((NC_CORE + P - 1) // P) * P   # 12544
PER_CORE = NV_SLOT + NC_SLOT        # 25088
N_PAD = N_CORES * PER_CORE          # 200704
NSB = PER_CORE // SB                # 98 superbands
NSB_VAR = NV_SLOT // SB             # 49 (sbs 0..48 var, 49..97 con)

GS = [10] * 9 + [8]                 # gather stripe sizes (superbands)
SUBS = [[10]] * 9 + [[8]]           # og sub-blocks per stripe
MSUB = max(max(s) for s in SUBS)    # max sub width (superbands)
# x replication pieces: (slot row0, rows per core). Each piece is one
# AllGather into a [8*rows, D] tensor; gather chunks are consecutive
# 25088-row slices of the concatenated piece tensors, 8 chunks total.
PIECES = [(0, 12544), (12544, 6272), (18816, 3136), (21952, 3136)]
_CH = []
for _pz, (_r0, _rp) in enumerate(PIECES):
    assert (N_CORES * _rp) % PER_CORE == 0
    for _j in range((N_CORES * _rp) // PER_CORE):
        _CH.append((_pz, _j))
assert len(_CH) == N_CORES
NCHUNK = N_CORES

BF = ml_dtypes.bfloat16


def _bf(a):
    return np.asarray(a, np.float32).astype(BF)


def _pack_nodes(deg_chunk, nsb_zero=1):
    """Assign node-local-ids (0..n-1) to slots within 49 superbands.

    deg_chunk: [n, 8] per-chunk in-degree. Returns slot_of_node [n].
    Last `nsb_zero` superbands get only zero-degree nodes (if enough),
    remaining nodes snake-dealt by total degree into the other sbs.
    """
    n = deg_chunk.shape[0]
    nsb = 49
    tot = deg_chunk.sum(axis=1)
    order = np.argsort(-tot, kind="stable")
    zero_ids = order[tot[order] == 0]
    nz_ids = order[tot[order] > 0]
    slot = np.full(n, -1, np.int64)
    # fill zero sbs from the back with zero-degree nodes
    zcap = nsb_zero * SB
    if len(zero_ids) < zcap:
        # not enough true-zero nodes: pull lowest-degree nodes too
        extra = nz_ids[len(nz_ids) - (zcap - len(zero_ids)):]
        nz_ids = nz_ids[:len(nz_ids) - (zcap - len(zero_ids))]
        zfill = np.concatenate([zero_ids, extra])
    else:
        zfill = zero_ids[:zcap]
        nz_ids = np.concatenate([nz_ids, zero_ids[zcap:]])
    slot[zfill] = (nsb - nsb_zero) * SB + np.arange(zcap)
    # greedy chunk-aware packing of nonzero nodes (degree-desc): place each
    # node in the sb minimizing the resulting max per-chunk count
    nact = nsb - nsb_zero
    k = len(nz_ids)
    cnt = np.zeros((nact, N_CORES), np.int64)
    used = np.zeros(nact, np.int64)
    sb_of = np.zeros(k, np.int64)
    degs = deg_chunk[nz_ids]
    for i in range(k):
        dn = degs[i]
        score = (cnt + dn).max(axis=1) * 1024 + used
        score[used >= SB] = 1 << 60
        s = int(np.argmin(score))
        sb_of[i] = s
        cnt[s] += dn
        used[s] += 1
    # rank within sb = order of assignment
    rank = np.zeros(k, np.int64)
    srt = np.argsort(sb_of, kind="stable")
    sb_sorted = sb_of[srt]
    starts = np.searchsorted(sb_sorted, np.arange(nact))
    ends = np.searchsorted(sb_sorted, np.arange(nact), side="right")
    for s in range(nact):
        sel = srt[starts[s]:ends[s]]
        rank[sel] = np.arange(ends[s] - starts[s])
    assert (rank < SB).all(), "superband overflow in packing"
    slot[nz_ids] = sb_of * SB + rank
    assert (slot >= 0).all()
    return slot


def _preprocess(inputs):
    vf = np.ascontiguousarray(np.asarray(inputs["var_node_features"], np.float32))
    cf = np.ascontiguousarray(np.asarray(inputs["con_node_features"], np.float32))
    ei = np.asarray(inputs["edge_index"])
    et = np.asarray(inputs["edge_types"]).astype(np.int64)
    assert (np.asarray(inputs["assoc_var"]) == np.arange(N_VAR)).all()
    assert (np.asarray(inputs["assoc_con"]) == N_VAR + np.arange(N_CON)).all()

    src = ei[0].astype(np.int64)
    dst = ei[1].astype(np.int64)

    # node -> (core, kind, local id): var v -> core v//NV_CORE, local v%NV_CORE
    node = np.arange(N_VAR + N_CON)
    isv_n = node < N_VAR
    core_n = np.where(isv_n, node // NV_CORE, (node - N_VAR) // NC_CORE)
    loc_n = np.where(isv_n, node % NV_CORE, (node - N_VAR) % NC_CORE)

    # per-node per-src-chunk in-degree (chunk = src core after permute —
    # but chunk of an edge depends on the SRC node's core, known already)
    src_core = np.where(src < N_VAR, src // NV_CORE, (src - N_VAR) // NC_CORE)
    # in-degree per (dst node, src core)
    degc = np.zeros((N_VAR + N_CON, N_CORES), np.int64)
    np.add.at(degc, (dst, src_core), 1)

    # pack per core: var + con separately; iterate with src-piece-aware
    # degree vectors (gather chunk = slot-range piece) and keep the best
    def pack_all(dc):
        slot = np.zeros(N_VAR + N_CON, np.int64)
        for c in range(N_CORES):
            vids = np.arange(c * NV_CORE, (c + 1) * NV_CORE)
            slot[vids] = _pack_nodes(dc[vids], nsb_zero=1)
            cids = N_VAR + np.arange(c * NC_CORE, (c + 1) * NC_CORE)
            slot[cids] = NV_SLOT + _pack_nodes(dc[cids], nsb_zero=1)
        return slot

    _row0s = np.array([r0 for r0, _ in PIECES] + [PER_CORE])
    _rps = np.array([rp for _, rp in PIECES])
    _base = np.cumsum([0] + [(N_CORES * rp) // PER_CORE for _, rp in PIECES])

    def chunk_of(pm_src):
        sl = pm_src % PER_CORE
        c = pm_src // PER_CORE
        pz = np.searchsorted(_row0s, sl, side="right") - 1
        grow = c * _rps[pz] + (sl - _row0s[pz])   # row in piece tensor
        ch = _base[pz] + grow // PER_CORE
        ls = grow % PER_CORE
        return ch, ls

    def t2_of(slot):
        pm = core_n * PER_CORE + slot
        pc, _ = chunk_of(pm[src])
        sb_ = (pm[dst] % PER_CORE) // SB
        sg = ((pm[dst] // PER_CORE) * NSB + sb_) * NCHUNK + pc
        cn = np.bincount(sg, minlength=N_CORES * NSB * NCHUNK).reshape(
            N_CORES, NSB, NCHUNK)
        return int(np.ceil(cn.max(axis=0) / P).sum())

    slot_of_node = pack_all(degc)
    best = (t2_of(slot_of_node), slot_of_node)
    cur = slot_of_node
    for _ in range(2):
        pm = core_n * PER_CORE + cur
        piece_lbl, _ = chunk_of(pm[src])
        degc2 = np.zeros((N_VAR + N_CON, NCHUNK), np.int64)
        np.add.at(degc2, (dst, piece_lbl), 1)
        cur = pack_all(degc2)
        t2v = t2_of(cur)
        if t2v < best[0]:
            best = (t2v, cur)
    slot_of_node = best[1]

    perm = core_n * PER_CORE + slot_of_node    # node -> padded row
    psrc = perm[src]
    pdst = perm[dst]
    cnt = np.bincount(pdst, minlength=N_PAD).astype(np.float32)
    invc = 1.0 / np.maximum(cnt, 1.0)
    atts = [np.asarray(inputs[f"att{l}"], np.float32)[:, 0] for l in (1, 2, 3)]

    core = pdst // PER_CORE
    sb = (pdst % PER_CORE) // SB          # superband 0..97
    dl2 = (pdst % SB).astype(np.float32)  # dst offset in window, 0..255
    chunk, lsrc64 = chunk_of(psrc)
    lsrc = lsrc64.astype(np.int16)

    # tiles per (superband, chunk): shared across cores (max)
    seg = (core * NSB + sb) * N_CORES + chunk
    counts = np.bincount(seg, minlength=N_CORES * NSB * N_CORES).reshape(
        N_CORES, NSB, N_CORES)
    tpb2 = np.ceil(counts.max(axis=0) / P).astype(np.int64)  # [NSB, CHUNKS]

    # canonical tile-column order: stripe G -> chunk c -> sb in G -> tiles
    assert sum(GS) == NSB
    groups = []
    s0 = 0
    for gsz in GS:
        groups.append(list(range(s0, s0 + gsz)))
        s0 += gsz
    colmap = {}
    ncol = 0
    for g, sbs in enumerate(groups):
        for c in range(N_CORES):
            for s in sbs:
                colmap[(s, c)] = ncol
                ncol += int(tpb2[s, c])
    T2 = ncol

    # rank of each edge within its (core, sb, chunk) run
    order = np.argsort(seg, kind="stable")
    seg_s = seg[order]
    run_starts = np.concatenate(
        [[0], np.cumsum(np.bincount(seg_s, minlength=seg.max() + 1))[:-1]])
    rank = np.arange(len(order)) - run_starts[seg_s]
    core_s = core[order]
    col0 = np.array([colmap[(s, c)] for s, c in
                     zip(sb[order].tolist(), chunk[order].tolist())])
    tcol = col0 + rank // P
    prow = rank % P

    lsrc_arr = np.zeros((N_CORES, P, T2), np.int16)
    dl_arr = np.zeros((N_CORES, P, T2), np.float32)
    vw_arr = np.zeros((3, N_CORES, P, T2), np.float32)
    lsrc_arr[core_s, prow, tcol] = lsrc[order]
    dl_arr[core_s, prow, tcol] = dl2[order]
    for li in range(3):
        vw_arr[li, core_s, prow, tcol] = atts[li][et[order]] * invc[pdst[order]]

    # int16 idxs in dma_gather layout: for linear j in a call, value at
    # partition 16*r + j%16 (replicated r=0..7), column tcol*8 + prow//16.
    idx16 = np.zeros((N_CORES, P, T2 * 8), np.int16)
    tc_all = np.repeat(np.arange(T2), P)
    pr_all = np.tile(np.arange(P), T2)
    cols = tc_all * 8 + pr_all // 16
    rows_ = pr_all % 16
    for cidx in range(N_CORES):
        vals = lsrc_arr[cidx, pr_all, tc_all]
        for r in range(8):
            idx16[cidx, 16 * r + rows_, cols] = vals

    # per-(stripe,chunk) call shapes
    call_tiles = []
    for g, sbs in enumerate(groups):
        call_tiles.append(
            [int(sum(int(tpb2[s, c]) for s in sbs)) for c in range(N_CORES)])

    iota256 = np.broadcast_to(
        np.arange(SB, dtype=np.float32), (P, SB)).astype(BF)
    ident = np.eye(P, dtype=np.float32).astype(BF)

    # per-core permuted features (staged per slot)
    in_maps = []
    for c in range(N_CORES):
        vfeat = np.zeros((NV_SLOT, 2), np.float32)
        vids = np.arange(c * NV_CORE, (c + 1) * NV_CORE)
        vfeat[slot_of_node[vids]] = vf[vids]
        cfeat = np.zeros((NC_SLOT, 2), np.float32)
        cids = np.arange(c * NC_CORE, (c + 1) * NC_CORE)
        cfeat[slot_of_node[N_VAR + cids] - NV_SLOT] = cf[cids]
        m = {
            "vfeatT": _bf(vfeat.T),
            "cfeatT": _bf(cfeat.T),
            "idx16": idx16[c],
            "dl2": dl_arr[c],
            "vw1": vw_arr[0, c],
            "vw2": vw_arr[1, c],
            "vw3": vw_arr[2, c],
            "iota256": iota256,
            "ident": ident,
            "fc1_w": _bf(inputs["fc1_w"]),
            "fc1_b": np.asarray(inputs["fc1_b"], np.float32),
            "fc4_w": _bf(inputs["fc4_w"]),
            "fc4_b": np.broadcast_to(
                np.asarray(inputs["fc4_b"], np.float32).reshape(1, 1),
                (P, 1)).copy(),
        }
        for t in ("var", "con"):
            m[f"{t}_w1"] = _bf(inputs[f"{t}_w1"])
            m[f"{t}_b1"] = np.asarray(inputs[f"{t}_b1"], np.float32)
            m[f"{t}_w2"] = _bf(inputs[f"{t}_w2"])
            m[f"{t}_b2"] = np.asarray(inputs[f"{t}_b2"], np.float32)
        for l in (1, 2, 3):
            m[f"basis{l}"] = _bf(np.asarray(inputs[f"basis{l}"],
                                            np.float32).reshape(D, D))
            m[f"root{l}"] = _bf(inputs[f"root{l}"])
            m[f"bias{l}"] = np.asarray(inputs[f"bias{l}"], np.float32)
        in_maps.append(m)
    # var slot -> global var id map for output assembly
    var_slot = slot_of_node[:N_VAR]    # slot within core var block
    key = tuple(int(t) for t in tpb2.reshape(-1))
    return in_maps, tpb2, groups, call_tiles, T2, var_slot, key


def _build_program(tpb2, groups, call_tiles, T2):
    nc = bacc.Bacc("TRN2", target_bir_lowering=False, debug=False,
                   num_devices=N_CORES, num_swdge_queues=4)

    def inp(name, shape, dtype=BF16):
        return nc.dram_tensor(name, shape, dtype, kind="ExternalInput")

    vfeatT = inp("vfeatT", [2, NV_SLOT])
    cfeatT = inp("cfeatT", [2, NC_SLOT])
    idx16 = inp("idx16", [P, T2 * 8], I16)
    dl2 = inp("dl2", [P, T2], F32)
    vws = {l: inp(f"vw{l}", [P, T2], F32) for l in (1, 2, 3)}
    iota256 = inp("iota256", [P, SB])
    identt = inp("ident", [P, P])
    mlp_w = {}
    for t in ("var", "con"):
        mlp_w[t] = (inp(f"{t}_w1", [2, D]), inp(f"{t}_b1", [D], F32),
                    inp(f"{t}_w2", [D, D]), inp(f"{t}_b2", [D], F32))
    rg_w = {l: (inp(f"basis{l}", [D, D]), inp(f"root{l}", [D, D]),
                inp(f"bias{l}", [D], F32)) for l in (1, 2, 3)}
    fc1_w = inp("fc1_w", [4 * D, D])
    fc1_b = inp("fc1_b", [D], F32)
    fc4_w = inp("fc4_w", [D, 1])
    fc4_b = inp("fc4_b", [P, 1], F32)
    y_out = nc.dram_tensor("y_out", [NV_SLOT], F32, kind="ExternalOutput")

    x_p = [[nc.dram_tensor(f"x{i}_p{p}", [N_CORES * rp, D], BF16,
                           kind="Internal", addr_space="Shared")
            for p, (_, rp) in enumerate(PIECES)] for i in range(3)]
    ag_in = [nc.dram_tensor(f"ag_in{i}", [PER_CORE, D], BF16, kind="Internal")
             for i in range(3)]
    xT_own = [nc.dram_tensor(f"xT{i}_own", [D, PER_CORE], BF16,
                             kind="Internal") for i in range(4)]

    rgroups = [list(range(N_CORES))]
    grp_cstart = []
    grp_base = []
    base = 0
    for g in range(len(groups)):
        cst = []
        off = 0
        for c in range(N_CORES):
            cst.append(off)
            off += call_tiles[g][c]
        grp_cstart.append(cst)
        grp_base.append(base)
        base += off
    max_grp_tiles = max(sum(ct) for ct in call_tiles)

    with tile.TileContext(nc) as tc:
        with tc.tile_pool(name="wp", bufs=1) as wp:
            nc.gpsimd.load_library(library_config.mlp)
            iota_t = wp.tile([P, SB], BF16, name="iota_t")
            nc.sync.dma_start(iota_t[:], iota256[:])
            ident = wp.tile([P, P], BF16, name="ident")
            nc.sync.dma_start(ident[:], identt[:])

            idx_sb = wp.tile([P, T2 * 8], I16, name="idx_sb")
            nc.sync.dma_start(idx_sb[:], idx16[:])
            dl_sb = wp.tile([P, T2], F32, name="dl_sb")
            nc.sync.dma_start(dl_sb[:], dl2[:])
            vw_sb = {}
            for l in (1, 2, 3):
                vw_sb[l] = wp.tile([P, T2], F32, name=f"vw_sb{l}")
                nc.sync.dma_start(vw_sb[l][:], vws[l][:])

            # ---------- phase A: input MLPs -> x0 (pairs of bands) ----------
            with tc.tile_pool(name="pa_f", bufs=1) as fp, \
                 tc.tile_pool(name="pa_sb", bufs=3) as sp, \
                 tc.tile_pool(name="pa_o", bufs=4) as op_a, \
                 tc.tile_pool(name="pa_pst", bufs=2, space="PSUM") as pp_t, \
                 tc.tile_pool(name="pa_ps", bufs=2, space="PSUM") as pp:
                pa_pieces = 0
                for ttype, featT, slot0, nslot in (
                    ("var", vfeatT, 0, NV_SLOT),
                    ("con", cfeatT, NV_SLOT, NC_SLOT),
                ):
                    w1, b1, w2, b2 = mlp_w[ttype]
                    w1s = wp.tile([2, D], BF16, name=f"w1s_{ttype}")
                    nc.sync.dma_start(w1s[:], w1[:])
                    b1s = wp.tile([P, 1], F32, name=f"b1s_{ttype}")
                    nc.sync.dma_start(b1s[:], b1.rearrange("(p one) -> p one", one=1))
                    w2s = wp.tile([D, D], BF16, name=f"w2s_{ttype}")
                    nc.sync.dma_start(w2s[:], w2[:])
                    b2s = wp.tile([P, 1], F32, name=f"b2s_{ttype}")
                    nc.sync.dma_start(b2s[:], b2.rearrange("(p one) -> p one", one=1))
                    ftT = fp.tile([2, nslot], BF16, name=f"ftT_{ttype}")
                    nc.sync.dma_start(ftT[:], featT[:])
                    npairs = nslot // SB
                    for j in range(npairs):
                        p1 = pp.tile([P, SB], F32, name="p1", space="PSUM")
                        nc.tensor.matmul(p1[:], lhsT=w1s[:],
                                         rhs=ftT[:, j * SB:(j + 1) * SB],
                                         start=True, stop=True)
                        h1 = sp.tile([P, SB], BF16, name="h1")
                        nc.scalar.activation(h1[:], p1[:],
                                             mybir.ActivationFunctionType.Relu,
                                             bias=b1s[:, :1])
                        p2 = pp.tile([P, SB], F32, name="p2", space="PSUM")
                        nc.tensor.matmul(p2[:], lhsT=w2s[:], rhs=h1[:],
                                         start=True, stop=True)
                        xTg = op_a.tile([P, SB], BF16, name="xTg")
                        nc.vector.tensor_scalar(
                            xTg[:], p2[:],
                            b2s[:, :1], None, op0=mybir.AluOpType.add)
                        rowg = op_a.tile([P, SB], BF16, name="rowg")
                        for h in range(2):
                            tp = pp_t.tile([P, P], BF16, name="tp",
                                           space="PSUM")
                            nc.tensor.transpose(
                                tp[:], xTg[:, h * P:(h + 1) * P], ident[:])
                            nc.scalar.activation(
                                rowg[:, h * P:(h + 1) * P], tp[:],
                                mybir.ActivationFunctionType.Copy)
                        col0 = slot0 + j * SB
                        nc.sync.dma_start(
                            xT_own[0][:, col0:col0 + SB], xTg[:])
                        nc.sync.dma_start(
                            ag_in[0][col0:col0 + SB, :]
                            .rearrange("(t p) d -> p t d", p=P),
                            rowg[:].rearrange("p (t d) -> p t d", d=D))
                        while (pa_pieces < len(PIECES)
                               and col0 + SB >= (PIECES[pa_pieces][0]
                                                 + PIECES[pa_pieces][1])):
                            pz = pa_pieces
                            r0, rp = PIECES[pz]
                            nc.gpsimd.collective_compute(
                                "AllGather", mybir.AluOpType.bypass,
                                replica_groups=rgroups,
                                ins=[ag_in[0][r0:r0 + rp, :]],
                                outs=[x_p[0][pz][:]])
                            pa_pieces += 1

            # head-MLP weights (consumed inside layer 3)
            fc1c = []
            for hl in range(4):
                t = wp.tile([D, D], BF16, name=f"fc1c{hl}")
                nc.sync.dma_start(t[:], fc1_w[hl * D:(hl + 1) * D, :])
                fc1c.append(t)
            fb1 = wp.tile([P, 1], F32, name="fb1")
            nc.sync.dma_start(fb1[:], fc1_b.rearrange("(p one) -> p one", one=1))
            f4w = wp.tile([D, 1], BF16, name="f4w")
            nc.sync.dma_start(f4w[:], fc4_w[:])
            f4b = wp.tile([P, 1], F32, name="f4b")
            nc.sync.dma_start(f4b[:], fc4_b[:])

            # ---------- phases B: 3 RGCN layers ----------
            for l in (1, 2, 3):
                basis, root, bias = rg_w[l]
                Bs = wp.tile([D, D], BF16, name=f"Bs_{l}")
                nc.sync.dma_start(Bs[:], basis[:])
                Rs = wp.tile([D, D], BF16, name=f"Rs_{l}")
                nc.sync.dma_start(Rs[:], root[:])
                bs = wp.tile([P, 1], F32, name=f"bs_{l}")
                nc.sync.dma_start(bs[:], bias.rearrange("(p one) -> p one", one=1))
                xcur = x_p[l - 1]
                xTc = xT_own[l - 1]
                xTn = xT_own[l]
                vwl = vw_sb[l]
                from contextlib import ExitStack
                with ExitStack() as stack:
                    gp = stack.enter_context(
                        tc.tile_pool(name=f"l{l}_g", bufs=3))
                    sp = stack.enter_context(
                        tc.tile_pool(name=f"l{l}_sb", bufs=3))
                    og = stack.enter_context(
                        tc.tile_pool(name=f"l{l}_og", bufs=2))
                    op = stack.enter_context(
                        tc.tile_pool(name=f"l{l}_o", bufs=24))
                    pp = stack.enter_context(
                        tc.tile_pool(name=f"l{l}_ps", bufs=4 if l < 3 else 2,
                                     space="PSUM"))
                    pq = stack.enter_context(
                        tc.tile_pool(name=f"l{l}_p2", bufs=2, space="PSUM"))
                    if l < 3:
                        pt = stack.enter_context(
                            tc.tile_pool(name=f"l{l}_pt", bufs=2,
                                         space="PSUM"))
                    else:
                        pt = stack.enter_context(
                            tc.tile_pool(name=f"l{l}_ph", bufs=2,
                                         space="PSUM"))
                        py = stack.enter_context(
                            tc.tile_pool(name=f"l{l}_py", bufs=2,
                                         space="PSUM"))
                    ly_pieces = 0
                    for g, sbs in enumerate(groups):
                        gb = grp_base[g]
                        xg = gp.tile([P, max_grp_tiles * P], BF16, name="xg")
                        for c in range(N_CORES):
                            nt = call_tiles[g][c]
                            if nt == 0:
                                continue
                            cs = grp_cstart[g][c]
                            nidx = nt * P
                            pz, jj = _CH[c][0], _CH[c][1]
                            nc.gpsimd.dma_gather(
                                xg[:, cs * P:(cs + nt) * P].rearrange(
                                    "p (t e) -> p t e", e=D),
                                xcur[pz][jj * PER_CORE:(jj + 1) * PER_CORE, :],
                                idx_sb[:, (gb + cs) * 8:(gb + cs + nt) * 8],
                                nidx, nidx, D, single_packet=False,
                                queue_num=c % 4)
                        sub0 = 0
                        for sub_n in SUBS[g]:
                            sub = sbs[sub0:sub0 + sub_n]
                            s_base = sub[0]
                            MS = MSUB * SB
                            xtbg = og.tile([P, MS], BF16, name="xtbg")
                            nc.sync.dma_start(
                                xtbg[:, :sub_n * SB],
                                xTc[:, s_base * SB:(s_base + sub_n) * SB])
                            outg = og.tile([P, MS], BF16, name="outg")
                            if l < 3:
                                rowg = og.tile([P, MS], BF16, name="rowg")
                            is_var = l == 3 and s_base < NSB_VAR
                            if is_var:
                                gnv = sum(1 for s in sub if s < NSB_VAR)
                                xtls = []
                                for hl in range(3):
                                    xtl = og.tile([P, MS], BF16,
                                                  name=f"xtl{hl}")
                                    nc.sync.dma_start(
                                        xtl[:, :gnv * SB],
                                        xT_own[hl][:, s_base * SB:
                                                    (s_base + gnv) * SB])
                                    xtls.append(xtl)
                                ybuf = og.tile([P, 2 * MSUB], F32,
                                               name="ybuf")
                            for si, s in enumerate(sub):
                                so = (s - sbs[0])
                                cols = []
                                for c in range(N_CORES):
                                    if tpb2[s, c] == 0:
                                        continue
                                    c0 = grp_cstart[g][c] + sum(
                                        int(tpb2[s2, c]) for s2 in sbs[:so])
                                    for t in range(int(tpb2[s, c])):
                                        cols.append(c0 + t)
                                st = pp.tile([P, SB], F32, name="st",
                                             space="PSUM")
                                for ti, col in enumerate(cols):
                                    gcol = gb + col
                                    o = op.tile([P, SB], BF16, name="o")
                                    nc.vector.tensor_scalar(
                                        o[:], iota_t[:],
                                        dl_sb[:, gcol:gcol + 1],
                                        vwl[:, gcol:gcol + 1],
                                        op0=mybir.AluOpType.is_equal,
                                        op1=mybir.AluOpType.mult)
                                    nc.tensor.matmul(
                                        st[:],
                                        lhsT=xg[:, col * P:(col + 1) * P],
                                        rhs=o[:],
                                        start=(ti == 0),
                                        stop=(ti == len(cols) - 1))
                                p2 = pq.tile([P, SB], F32, name="p2",
                                             space="PSUM")
                                xtb = xtbg[:, si * SB:(si + 1) * SB]
                                if cols:
                                    sn = sp.tile([P, SB], BF16, name="sn")
                                    if l == 3:
                                        nc.vector.tensor_copy(sn[:], st[:])
                                    else:
                                        nc.scalar.activation(
                                            sn[:], st[:],
                                            mybir.ActivationFunctionType.Copy)
                                    nc.tensor.matmul(p2[:], lhsT=Bs[:],
                                                     rhs=sn[:],
                                                     start=True, stop=False)
                                    nc.tensor.matmul(
                                        p2[:], lhsT=Rs[:], rhs=xtb,
                                        start=False, stop=True)
                                else:
                                    nc.tensor.matmul(
                                        p2[:], lhsT=Rs[:], rhs=xtb,
                                        start=True, stop=True)
                                outs = outg[:, si * SB:(si + 1) * SB]
                                nc.scalar.activation(
                                    outs, p2[:],
                                    mybir.ActivationFunctionType.Relu,
                                    bias=bs[:, :1])
                                if l < 3:
                                    for h in range(2):
                                        c0 = si * SB + h * P
                                        tp2 = pt.tile([P, P], BF16,
                                                      name="tp2",
                                                      space="PSUM")
                                        nc.tensor.transpose(
                                            tp2[:], outg[:, c0:c0 + P],
                                            ident[:])
                                        nc.scalar.activation(
                                            rowg[:, c0:c0 + P], tp2[:],
                                            mybir.ActivationFunctionType.Copy)
                                if is_var and s < NSB_VAR:
                                    for h in range(2):
                                        cc0 = si * SB + h * P
                                        hp = pt.tile([P, P], F32, name="hp",
                                                     space="PSUM")
                                        for hl in range(3):
                                            nc.tensor.matmul(
                                                hp[:], lhsT=fc1c[hl][:],
                                                rhs=xtls[hl][:, cc0:cc0 + P],
                                                start=(hl == 0), stop=False)
                                        nc.tensor.matmul(
                                            hp[:], lhsT=fc1c[3][:],
                                            rhs=outg[:, cc0:cc0 + P],
                                            start=False, stop=True)
                                        hr = sp.tile([P, P], BF16, name="hr")
                                        nc.vector.tensor_scalar(
                                            hr[:], hp[:], fb1[:, :1], 0.0,
                                            op0=mybir.AluOpType.add,
                                            op1=mybir.AluOpType.max)
                                        yp = py.tile([P, 1], F32, name="yp",
                                                     space="PSUM")
                                        nc.tensor.matmul(yp[:], lhsT=hr[:],
                                                         rhs=f4w[:],
                                                         start=True,
                                                         stop=True)
                                        nc.vector.tensor_scalar(
                                            ybuf[:, si * 2 + h:si * 2 + h + 1],
                                            yp[:], f4b[:, :1], None,
                                            op0=mybir.AluOpType.add)
                            nc.sync.dma_start(
                                xTn[:, s_base * SB:(s_base + sub_n) * SB],
                                outg[:, :sub_n * SB])
                            if l < 3:
                                r0 = s_base * SB
                                nc.sync.dma_start(
                                    ag_in[l - 1][r0:r0 + sub_n * SB, :]
                                    .rearrange("(t p) d -> p t d", p=P),
                                    rowg[:, :sub_n * SB].rearrange(
                                        "p (t d) -> p t d", d=D))
                            if is_var:
                                nc.sync.dma_start(
                                    y_out[s_base * SB:(s_base + gnv) * SB]
                                    .rearrange("(g p) -> p g", p=P),
                                    ybuf[:, :gnv * 2])
                            sub0 += sub_n
                            if l < 3:
                                rows_done = (s_base + sub_n) * SB
                                while (ly_pieces < len(PIECES)
                                       and rows_done
                                       >= (PIECES[ly_pieces][0]
                                           + PIECES[ly_pieces][1])):
                                    pz = ly_pieces
                                    r0, rp = PIECES[pz]
                                    nc.gpsimd.collective_compute(
                                        "AllGather", mybir.AluOpType.bypass,
                                        replica_groups=rgroups,
                                        ins=[ag_in[l - 1][r0:r0 + rp, :]],
                                        outs=[x_p[l][pz][:]])
                                    ly_pieces += 1


    nc.compile()
    return nc


_CACHE = {}


def kernel(**inputs) -> np.ndarray:
    from concourse import bass_utils
    in_maps, tpb2, groups, call_tiles, T2, var_slot, key = _preprocess(inputs)
    if key not in _CACHE:
        _CACHE[key] = _build_program(tpb2, groups, call_tiles, T2)
    nc = _CACHE[key]
    res = bass_utils.run_bass_kernel_spmd(
        nc, in_maps, core_ids=list(range(N_CORES)))
    y = np.empty(N_VAR, np.float32)
    for c in range(N_CORES):
        ys = res.results[c]["y_out"]
        vids = np.arange(c * NV_CORE, (c + 1) * NV_CORE)
        y[vids] = ys[var_slot[vids]]
    return y.astype(np.float32)


# revision 53
# speedup vs baseline: 1.2149x; 1.0496x over previous
"""Trainium2 Bass kernel for nn_Net_69114613727316 (RGCN message passing).

Self-contained: kernel(**inputs) -> np.ndarray [100000] float32.

Math (exploiting num_bases=1): w[r] = att[r,0] * basis, so
    agg_d = ((sum_e v_e * x[src_e]) @ basis),  v_e = att_l[etype_e]/max(cnt_d,1)
    out = relu(agg + x @ root + bias)
Per-edge GEMMs collapse into a weighted scatter-add (one-hot matmuls into
PSUM per 256-dst "superband" window) plus one dense GEMM per window.

Sharding: node space is permuted so core c owns rows
[c*25088, (c+1)*25088) = [12544 var slots | 12544 con slots].  Each
layer's full x (bf16) is replicated via piece-wise AllGather so any core
can gather arbitrary src rows; each core computes only its own rows.

v6 (from v5):
  * degree-balanced node->slot packing per core: per-(superband, chunk)
    edge counts equalized so nearly every (sb, chunk) needs exactly one
    128-edge tile (T2 ~790 vs 822); zero-in-degree nodes concentrated in
    a dedicated superband per type whose scatter is skipped entirely
  * piece-wise AllGather: x is replicated in 4 slot-range pieces
    (12544/6272/3136/3136 rows) launched as soon as their rows are
    computed, so collective latency overlaps layer compute; gather
    chunks are 25088-row slices of the concatenated piece tensors
  * 10-superband gather stripes with 3-deep xg buffering keep all 4
    SWDGE queues fed ahead of the PE
  * phase-A input MLPs processed in 256-column pairs; layer-3 PSUM->SBUF
    copies and head relu moved from ACT to the idle DVE
"""
import numpy as np
import ml_dtypes

import concourse.bass as bass
import concourse.bacc as bacc
import concourse.tile as tile
import concourse.mybir as mybir
from concourse import library_config

F32 = mybir.dt.float32
BF16 = mybir.dt.bfloat16
I16 = mybir.dt.int16
D = 128
P = 128
SB = 256                      # superband width (dst window)

N_VAR = 100000
N_CON = 100000
N_EDGES = 640000
N_CORES = 8

NV_CORE = N_VAR // N_CORES          # 12500
NC_CORE = N_CON // N_CORES          # 12500
NV_SLOT = ((NV_CORE + P - 1) // P) * P   # 12544
NC_SLOT = ((NC_CORE + P - 1) // P) * P   # 12544
PER_CORE = NV_SLOT + NC_SLOT        # 25088
N_PAD = N_CORES * PER_CORE          # 200704
NSB = PER_CORE // SB                # 98 superbands
NSB_VAR = NV_SLOT // SB             # 49 (sbs 0..48 var, 49..97 con)

GS = [10] * 9 + [8]                 # gather stripe sizes (superbands)
SUBS = [[10]] * 9 + [[8]]           # og sub-blocks per stripe
MSUB = max(max(s) for s in SUBS)    # max sub width (superbands)
# x replication pieces: (slot row0, rows per core). Each piece is one
# AllGather into a [8*rows, D] tensor; gather chunks are consecutive
# 25088-row slices of the concatenated piece tensors, 8 chunks total.
PIECES = [(0, 12544), (12544, 6272), (18816, 6272)]
_CH = []
for _pz, (_r0, _rp) in enumerate(PIECES):
    assert (N_CORES * _rp) % PER_CORE == 0
    for _j in range((N_CORES * _rp) // PER_CORE):
        _CH.append((_pz, _j))
assert len(_CH) == N_CORES
NCHUNK = N_CORES

BF = ml_dtypes.bfloat16


def _bf(a):
    return np.asarray(a, np.float32).astype(BF)


def _pack_nodes(deg_chunk, nsb_zero=1):
    """Assign node-local-ids (0..n-1) to slots within 49 superbands.

    deg_chunk: [n, 8] per-chunk in-degree. Returns slot_of_node [n].
    Last `nsb_zero` superbands get only zero-degree nodes (if enough),
    remaining nodes snake-dealt by total degree into the other sbs.
    """
    n = deg_chunk.shape[0]
    nsb = 49
    tot = deg_chunk.sum(axis=1)
    order = np.argsort(-tot, kind="stable")
    zero_ids = order[tot[order] == 0]
    nz_ids = order[tot[order] > 0]
    slot = np.full(n, -1, np.int64)
    # fill zero sbs from the back with zero-degree nodes
    zcap = nsb_zero * SB
    if len(zero_ids) < zcap:
        # not enough true-zero nodes: pull lowest-degree nodes too
        extra = nz_ids[len(nz_ids) - (zcap - len(zero_ids)):]
        nz_ids = nz_ids[:len(nz_ids) - (zcap - len(zero_ids))]
        zfill = np.concatenate([zero_ids, extra])
    else:
        zfill = zero_ids[:zcap]
        nz_ids = np.concatenate([nz_ids, zero_ids[zcap:]])
    slot[zfill] = (nsb - nsb_zero) * SB + np.arange(zcap)
    # greedy chunk-aware packing of nonzero nodes (degree-desc): place each
    # node in the sb minimizing the resulting max per-chunk count
    nact = nsb - nsb_zero
    k = len(nz_ids)
    cnt = np.zeros((nact, N_CORES), np.int64)
    used = np.zeros(nact, np.int64)
    sb_of = np.zeros(k, np.int64)
    degs = deg_chunk[nz_ids]
    for i in range(k):
        dn = degs[i]
        score = (cnt + dn).max(axis=1) * 1024 + used
        score[used >= SB] = 1 << 60
        s = int(np.argmin(score))
        sb_of[i] = s
        cnt[s] += dn
        used[s] += 1
    # rank within sb = order of assignment
    rank = np.zeros(k, np.int64)
    srt = np.argsort(sb_of, kind="stable")
    sb_sorted = sb_of[srt]
    starts = np.searchsorted(sb_sorted, np.arange(nact))
    ends = np.searchsorted(sb_sorted, np.arange(nact), side="right")
    for s in range(nact):
        sel = srt[starts[s]:ends[s]]
        rank[sel] = np.arange(ends[s] - starts[s])
    assert (rank < SB).all(), "superband overflow in packing"
    slot[nz_ids] = sb_of * SB + rank
    assert (slot >= 0).all()
    return slot


def _preprocess(inputs):
    vf = np.ascontiguousarray(np.asarray(inputs["var_node_features"], np.float32))
    cf = np.ascontiguousarray(np.asarray(inputs["con_node_features"], np.float32))
    ei = np.asarray(inputs["edge_index"])
    et = np.asarray(inputs["edge_types"]).astype(np.int64)
    assert (np.asarray(inputs["assoc_var"]) == np.arange(N_VAR)).all()
    assert (np.asarray(inputs["assoc_con"]) == N_VAR + np.arange(N_CON)).all()

    src = ei[0].astype(np.int64)
    dst = ei[1].astype(np.int64)

    # node -> (core, kind, local id): var v -> core v//NV_CORE, local v%NV_CORE
    node = np.arange(N_VAR + N_CON)
    isv_n = node < N_VAR
    core_n = np.where(isv_n, node // NV_CORE, (node - N_VAR) // NC_CORE)
    loc_n = np.where(isv_n, node % NV_CORE, (node - N_VAR) % NC_CORE)

    # per-node per-src-chunk in-degree (chunk = src core after permute —
    # but chunk of an edge depends on the SRC node's core, known already)
    src_core = np.where(src < N_VAR, src // NV_CORE, (src - N_VAR) // NC_CORE)
    # in-degree per (dst node, src core)
    degc = np.zeros((N_VAR + N_CON, N_CORES), np.int64)
    np.add.at(degc, (dst, src_core), 1)

    # pack per core: var + con separately; iterate with src-piece-aware
    # degree vectors (gather chunk = slot-range piece) and keep the best
    def pack_all(dc):
        slot = np.zeros(N_VAR + N_CON, np.int64)
        for c in range(N_CORES):
            vids = np.arange(c * NV_CORE, (c + 1) * NV_CORE)
            slot[vids] = _pack_nodes(dc[vids], nsb_zero=1)
            cids = N_VAR + np.arange(c * NC_CORE, (c + 1) * NC_CORE)
            slot[cids] = NV_SLOT + _pack_nodes(dc[cids], nsb_zero=1)
        return slot

    _row0s = np.array([r0 for r0, _ in PIECES] + [PER_CORE])
    _rps = np.array([rp for _, rp in PIECES])
    _base = np.cumsum([0] + [(N_CORES * rp) // PER_CORE for _, rp in PIECES])

    def chunk_of(pm_src):
        sl = pm_src % PER_CORE
        c = pm_src // PER_CORE
        pz = np.searchsorted(_row0s, sl, side="right") - 1
        grow = c * _rps[pz] + (sl - _row0s[pz])   # row in piece tensor
        ch = _base[pz] + grow // PER_CORE
        ls = grow % PER_CORE
        return ch, ls

    def t2_of(slot):
        pm = core_n * PER_CORE + slot
        pc, _ = chunk_of(pm[src])
        sb_ = (pm[dst] % PER_CORE) // SB
        sg = ((pm[dst] // PER_CORE) * NSB + sb_) * NCHUNK + pc
        cn = np.bincount(sg, minlength=N_CORES * NSB * NCHUNK).reshape(
            N_CORES, NSB, NCHUNK)
        return int(np.ceil(cn.max(axis=0) / P).sum())

    slot_of_node = pack_all(degc)
    best = (t2_of(slot_of_node), slot_of_node)
    cur = slot_of_node
    for _ in range(2):
        pm = core_n * PER_CORE + cur
        piece_lbl, _ = chunk_of(pm[src])
        degc2 = np.zeros((N_VAR + N_CON, NCHUNK), np.int64)
        np.add.at(degc2, (dst, piece_lbl), 1)
        cur = pack_all(degc2)
        t2v = t2_of(cur)
        if t2v < best[0]:
            best = (t2v, cur)
    slot_of_node = best[1]

    perm = core_n * PER_CORE + slot_of_node    # node -> padded row
    psrc = perm[src]
    pdst = perm[dst]
    cnt = np.bincount(pdst, minlength=N_PAD).astype(np.float32)
    invc = 1.0 / np.maximum(cnt, 1.0)
    atts = [np.asarray(inputs[f"att{l}"], np.float32)[:, 0] for l in (1, 2, 3)]

    core = pdst // PER_CORE
    sb = (pdst % PER_CORE) // SB          # superband 0..97
    dl2 = (pdst % SB).astype(np.float32)  # dst offset in window, 0..255
    chunk, lsrc64 = chunk_of(psrc)
    lsrc = lsrc64.astype(np.int16)

    # tiles per (superband, chunk): shared across cores (max)
    seg = (core * NSB + sb) * N_CORES + chunk
    counts = np.bincount(seg, minlength=N_CORES * NSB * N_CORES).reshape(
        N_CORES, NSB, N_CORES)
    tpb2 = np.ceil(counts.max(axis=0) / P).astype(np.int64)  # [NSB, CHUNKS]

    # canonical tile-column order: stripe G -> chunk c -> sb in G -> tiles
    assert sum(GS) == NSB
    groups = []
    s0 = 0
    for gsz in GS:
        groups.append(list(range(s0, s0 + gsz)))
        s0 += gsz
    colmap = {}
    ncol = 0
    for g, sbs in enumerate(groups):
        for c in range(N_CORES):
            for s in sbs:
                colmap[(s, c)] = ncol
                ncol += int(tpb2[s, c])
    T2 = ncol

    # rank of each edge within its (core, sb, chunk) run
    order = np.argsort(seg, kind="stable")
    seg_s = seg[order]
    run_starts = np.concatenate(
        [[0], np.cumsum(np.bincount(seg_s, minlength=seg.max() + 1))[:-1]])
    rank = np.arange(len(order)) - run_starts[seg_s]
    core_s = core[order]
    col0 = np.array([colmap[(s, c)] for s, c in
                     zip(sb[order].tolist(), chunk[order].tolist())])
    tcol = col0 + rank // P
    prow = rank % P

    lsrc_arr = np.zeros((N_CORES, P, T2), np.int16)
    dl_arr = np.zeros((N_CORES, P, T2), np.float32)
    vw_arr = np.zeros((3, N_CORES, P, T2), np.float32)
    lsrc_arr[core_s, prow, tcol] = lsrc[order]
    dl_arr[core_s, prow, tcol] = dl2[order]
    for li in range(3):
        vw_arr[li, core_s, prow, tcol] = atts[li][et[order]] * invc[pdst[order]]

    # int16 idxs in dma_gather layout: for linear j in a call, value at
    # partition 16*r + j%16 (replicated r=0..7), column tcol*8 + prow//16.
    idx16 = np.zeros((N_CORES, P, T2 * 8), np.int16)
    tc_all = np.repeat(np.arange(T2), P)
    pr_all = np.tile(np.arange(P), T2)
    cols = tc_all * 8 + pr_all // 16
    rows_ = pr_all % 16
    for cidx in range(N_CORES):
        vals = lsrc_arr[cidx, pr_all, tc_all]
        for r in range(8):
            idx16[cidx, 16 * r + rows_, cols] = vals

    # per-(stripe,chunk) call shapes
    call_tiles = []
    for g, sbs in enumerate(groups):
        call_tiles.append(
            [int(sum(int(tpb2[s, c]) for s in sbs)) for c in range(N_CORES)])

    iota256 = np.broadcast_to(
        np.arange(SB, dtype=np.float32), (P, SB)).astype(BF)
    ident = np.eye(P, dtype=np.float32).astype(BF)

    # per-core permuted features (staged per slot)
    in_maps = []
    for c in range(N_CORES):
        vfeat = np.zeros((NV_SLOT, 2), np.float32)
        vids = np.arange(c * NV_CORE, (c + 1) * NV_CORE)
        vfeat[slot_of_node[vids]] = vf[vids]
        cfeat = np.zeros((NC_SLOT, 2), np.float32)
        cids = np.arange(c * NC_CORE, (c + 1) * NC_CORE)
        cfeat[slot_of_node[N_VAR + cids] - NV_SLOT] = cf[cids]
        m = {
            "vfeatT": _bf(vfeat.T),
            "cfeatT": _bf(cfeat.T),
            "idx16": idx16[c],
            "dl2": dl_arr[c],
            "vw1": vw_arr[0, c],
            "vw2": vw_arr[1, c],
            "vw3": vw_arr[2, c],
            "iota256": iota256,
            "ident": ident,
            "fc1_w": _bf(inputs["fc1_w"]),
            "fc1_b": np.asarray(inputs["fc1_b"], np.float32),
            "fc4_w": _bf(inputs["fc4_w"]),
            "fc4_b": np.broadcast_to(
                np.asarray(inputs["fc4_b"], np.float32).reshape(1, 1),
                (P, 1)).copy(),
        }
        for t in ("var", "con"):
            m[f"{t}_w1"] = _bf(inputs[f"{t}_w1"])
            m[f"{t}_b1"] = np.asarray(inputs[f"{t}_b1"], np.float32)
            m[f"{t}_w2"] = _bf(inputs[f"{t}_w2"])
            m[f"{t}_b2"] = np.asarray(inputs[f"{t}_b2"], np.float32)
        for l in (1, 2, 3):
            m[f"basis{l}"] = _bf(np.asarray(inputs[f"basis{l}"],
                                            np.float32).reshape(D, D))
            m[f"root{l}"] = _bf(inputs[f"root{l}"])
            m[f"bias{l}"] = np.asarray(inputs[f"bias{l}"], np.float32)
        in_maps.append(m)
    # var slot -> global var id map for output assembly
    var_slot = slot_of_node[:N_VAR]    # slot within core var block
    key = tuple(int(t) for t in tpb2.reshape(-1))
    return in_maps, tpb2, groups, call_tiles, T2, var_slot, key


def _build_program(tpb2, groups, call_tiles, T2):
    nc = bacc.Bacc("TRN2", target_bir_lowering=False, debug=False,
                   num_devices=N_CORES, num_swdge_queues=4)

    def inp(name, shape, dtype=BF16):
        return nc.dram_tensor(name, shape, dtype, kind="ExternalInput")

    vfeatT = inp("vfeatT", [2, NV_SLOT])
    cfeatT = inp("cfeatT", [2, NC_SLOT])
    idx16 = inp("idx16", [P, T2 * 8], I16)
    dl2 = inp("dl2", [P, T2], F32)
    vws = {l: inp(f"vw{l}", [P, T2], F32) for l in (1, 2, 3)}
    iota256 = inp("iota256", [P, SB])
    identt = inp("ident", [P, P])
    mlp_w = {}
    for t in ("var", "con"):
        mlp_w[t] = (inp(f"{t}_w1", [2, D]), inp(f"{t}_b1", [D], F32),
                    inp(f"{t}_w2", [D, D]), inp(f"{t}_b2", [D], F32))
    rg_w = {l: (inp(f"basis{l}", [D, D]), inp(f"root{l}", [D, D]),
                inp(f"bias{l}", [D], F32)) for l in (1, 2, 3)}
    fc1_w = inp("fc1_w", [4 * D, D])
    fc1_b = inp("fc1_b", [D], F32)
    fc4_w = inp("fc4_w", [D, 1])
    fc4_b = inp("fc4_b", [P, 1], F32)
    y_out = nc.dram_tensor("y_out", [NV_SLOT], F32, kind="ExternalOutput")

    x_p = [[nc.dram_tensor(f"x{i}_p{p}", [N_CORES * rp, D], BF16,
                           kind="Internal", addr_space="Shared")
            for p, (_, rp) in enumerate(PIECES)] for i in range(3)]
    ag_in = [nc.dram_tensor(f"ag_in{i}", [PER_CORE, D], BF16, kind="Internal")
             for i in range(3)]
    xT_own = [nc.dram_tensor(f"xT{i}_own", [D, PER_CORE], BF16,
                             kind="Internal") for i in range(4)]

    rgroups = [list(range(N_CORES))]
    grp_cstart = []
    grp_base = []
    base = 0
    for g in range(len(groups)):
        cst = []
        off = 0
        for c in range(N_CORES):
            cst.append(off)
            off += call_tiles[g][c]
        grp_cstart.append(cst)
        grp_base.append(base)
        base += off
    max_grp_tiles = max(sum(ct) for ct in call_tiles)

    with tile.TileContext(nc) as tc:
        with tc.tile_pool(name="wp", bufs=1) as wp:
            nc.gpsimd.load_library(library_config.mlp)
            iota_t = wp.tile([P, SB], BF16, name="iota_t")
            nc.sync.dma_start(iota_t[:], iota256[:])
            ident = wp.tile([P, P], BF16, name="ident")
            nc.sync.dma_start(ident[:], identt[:])

            idx_sb = wp.tile([P, T2 * 8], I16, name="idx_sb")
            nc.sync.dma_start(idx_sb[:], idx16[:])
            dl_sb = wp.tile([P, T2], F32, name="dl_sb")
            nc.sync.dma_start(dl_sb[:], dl2[:])
            vw_sb = {}
            for l in (1, 2, 3):
                vw_sb[l] = wp.tile([P, T2], F32, name=f"vw_sb{l}")
                nc.sync.dma_start(vw_sb[l][:], vws[l][:])

            # ---------- phase A: input MLPs -> x0 (pairs of bands) ----------
            with tc.tile_pool(name="pa_f", bufs=1) as fp, \
                 tc.tile_pool(name="pa_sb", bufs=3) as sp, \
                 tc.tile_pool(name="pa_o", bufs=4) as op_a, \
                 tc.tile_pool(name="pa_pst", bufs=2, space="PSUM") as pp_t, \
                 tc.tile_pool(name="pa_ps", bufs=2, space="PSUM") as pp:
                pa_pieces = 0
                for ttype, featT, slot0, nslot in (
                    ("var", vfeatT, 0, NV_SLOT),
                    ("con", cfeatT, NV_SLOT, NC_SLOT),
                ):
                    w1, b1, w2, b2 = mlp_w[ttype]
                    w1s = wp.tile([2, D], BF16, name=f"w1s_{ttype}")
                    nc.sync.dma_start(w1s[:], w1[:])
                    b1s = wp.tile([P, 1], F32, name=f"b1s_{ttype}")
                    nc.sync.dma_start(b1s[:], b1.rearrange("(p one) -> p one", one=1))
                    w2s = wp.tile([D, D], BF16, name=f"w2s_{ttype}")
                    nc.sync.dma_start(w2s[:], w2[:])
                    b2s = wp.tile([P, 1], F32, name=f"b2s_{ttype}")
                    nc.sync.dma_start(b2s[:], b2.rearrange("(p one) -> p one", one=1))
                    ftT = fp.tile([2, nslot], BF16, name=f"ftT_{ttype}")
                    nc.sync.dma_start(ftT[:], featT[:])
                    npairs = nslot // SB
                    for j in range(npairs):
                        p1 = pp.tile([P, SB], F32, name="p1", space="PSUM")
                        nc.tensor.matmul(p1[:], lhsT=w1s[:],
                                         rhs=ftT[:, j * SB:(j + 1) * SB],
                                         start=True, stop=True)
                        h1 = sp.tile([P, SB], BF16, name="h1")
                        nc.scalar.activation(h1[:], p1[:],
                                             mybir.ActivationFunctionType.Relu,
                                             bias=b1s[:, :1])
                        p2 = pp.tile([P, SB], F32, name="p2", space="PSUM")
                        nc.tensor.matmul(p2[:], lhsT=w2s[:], rhs=h1[:],
                                         start=True, stop=True)
                        xTg = op_a.tile([P, SB], BF16, name="xTg")
                        nc.vector.tensor_scalar(
                            xTg[:], p2[:],
                            b2s[:, :1], None, op0=mybir.AluOpType.add)
                        rowg = op_a.tile([P, SB], BF16, name="rowg")
                        for h in range(2):
                            tp = pp_t.tile([P, P], BF16, name="tp",
                                           space="PSUM")
                            nc.tensor.transpose(
                                tp[:], xTg[:, h * P:(h + 1) * P], ident[:])
                            nc.scalar.activation(
                                rowg[:, h * P:(h + 1) * P], tp[:],
                                mybir.ActivationFunctionType.Copy)
                        col0 = slot0 + j * SB
                        nc.sync.dma_start(
                            xT_own[0][:, col0:col0 + SB], xTg[:])
                        nc.sync.dma_start(
                            ag_in[0][col0:col0 + SB, :]
                            .rearrange("(t p) d -> p t d", p=P),
                            rowg[:].rearrange("p (t d) -> p t d", d=D))
                        while (pa_pieces < len(PIECES)
                               and col0 + SB >= (PIECES[pa_pieces][0]
                                                 + PIECES[pa_pieces][1])):
                            pz = pa_pieces
                            r0, rp = PIECES[pz]
                            nc.gpsimd.collective_compute(
                                "AllGather", mybir.AluOpType.bypass,
                                replica_groups=rgroups,
                                ins=[ag_in[0][r0:r0 + rp, :]],
                                outs=[x_p[0][pz][:]])
                            pa_pieces += 1

            # head-MLP weights (consumed inside layer 3)
            fc1c = []
            for hl in range(4):
                t = wp.tile([D, D], BF16, name=f"fc1c{hl}")
                nc.sync.dma_start(t[:], fc1_w[hl * D:(hl + 1) * D, :])
                fc1c.append(t)
            fb1 = wp.tile([P, 1], F32, name="fb1")
            nc.sync.dma_start(fb1[:], fc1_b.rearrange("(p one) -> p one", one=1))
            f4w = wp.tile([D, 1], BF16, name="f4w")
            nc.sync.dma_start(f4w[:], fc4_w[:])
            f4b = wp.tile([P, 1], F32, name="f4b")
            nc.sync.dma_start(f4b[:], fc4_b[:])

            # ---------- phases B: 3 RGCN layers ----------
            for l in (1, 2, 3):
                basis, root, bias = rg_w[l]
                Bs = wp.tile([D, D], BF16, name=f"Bs_{l}")
                nc.sync.dma_start(Bs[:], basis[:])
                Rs = wp.tile([D, D], BF16, name=f"Rs_{l}")
                nc.sync.dma_start(Rs[:], root[:])
                bs = wp.tile([P, 1], F32, name=f"bs_{l}")
                nc.sync.dma_start(bs[:], bias.rearrange("(p one) -> p one", one=1))
                xcur = x_p[l - 1]
                xTc = xT_own[l - 1]
                xTn = xT_own[l]
                vwl = vw_sb[l]
                from contextlib import ExitStack
                with ExitStack() as stack:
                    gp = stack.enter_context(
                        tc.tile_pool(name=f"l{l}_g", bufs=3))
                    sp = stack.enter_context(
                        tc.tile_pool(name=f"l{l}_sb", bufs=3))
                    og = stack.enter_context(
                        tc.tile_pool(name=f"l{l}_og", bufs=2))
                    op = stack.enter_context(
                        tc.tile_pool(name=f"l{l}_o", bufs=24))
                    pp = stack.enter_context(
                        tc.tile_pool(name=f"l{l}_ps", bufs=4 if l < 3 else 2,
                                     space="PSUM"))
                    pq = stack.enter_context(
                        tc.tile_pool(name=f"l{l}_p2", bufs=2, space="PSUM"))
                    if l < 3:
                        pt = stack.enter_context(
                            tc.tile_pool(name=f"l{l}_pt", bufs=2,
                                         space="PSUM"))
                    else:
                        pt = stack.enter_context(
                            tc.tile_pool(name=f"l{l}_ph", bufs=2,
                                         space="PSUM"))
                        py = stack.enter_context(
                            tc.tile_pool(name=f"l{l}_py", bufs=2,
                                         space="PSUM"))
                    ly_pieces = 0
                    for g, sbs in enumerate(groups):
                        gb = grp_base[g]
                        xg = gp.tile([P, max_grp_tiles * P], BF16, name="xg")
                        for c in range(N_CORES):
                            nt = call_tiles[g][c]
                            if nt == 0:
                                continue
                            cs = grp_cstart[g][c]
                            nidx = nt * P
                            pz, jj = _CH[c][0], _CH[c][1]
                            nc.gpsimd.dma_gather(
                                xg[:, cs * P:(cs + nt) * P].rearrange(
                                    "p (t e) -> p t e", e=D),
                                xcur[pz][jj * PER_CORE:(jj + 1) * PER_CORE, :],
                                idx_sb[:, (gb + cs) * 8:(gb + cs + nt) * 8],
                                nidx, nidx, D, single_packet=False,
                                queue_num=c % 4)
                        sub0 = 0
                        for sub_n in SUBS[g]:
                            sub = sbs[sub0:sub0 + sub_n]
                            s_base = sub[0]
                            MS = MSUB * SB
                            xtbg = og.tile([P, MS], BF16, name="xtbg")
                            nc.sync.dma_start(
                                xtbg[:, :sub_n * SB],
                                xTc[:, s_base * SB:(s_base + sub_n) * SB])
                            outg = og.tile([P, MS], BF16, name="outg")
                            if l < 3:
                                rowg = og.tile([P, MS], BF16, name="rowg")
                            is_var = l == 3 and s_base < NSB_VAR
                            if is_var:
                                gnv = sum(1 for s in sub if s < NSB_VAR)
                                xtls = []
                                for hl in range(3):
                                    xtl = og.tile([P, MS], BF16,
                                                  name=f"xtl{hl}")
                                    nc.sync.dma_start(
                                        xtl[:, :gnv * SB],
                                        xT_own[hl][:, s_base * SB:
                                                    (s_base + gnv) * SB])
                                    xtls.append(xtl)
                                ybuf = og.tile([P, 2 * MSUB], F32,
                                               name="ybuf")
                            for si, s in enumerate(sub):
                                so = (s - sbs[0])
                                cols = []
                                for c in range(N_CORES):
                                    if tpb2[s, c] == 0:
                                        continue
                                    c0 = grp_cstart[g][c] + sum(
                                        int(tpb2[s2, c]) for s2 in sbs[:so])
                                    for t in range(int(tpb2[s, c])):
                                        cols.append(c0 + t)
                                st = pp.tile([P, SB], F32, name="st",
                                             space="PSUM")
                                for ti, col in enumerate(cols):
                                    gcol = gb + col
                                    o = op.tile([P, SB], BF16, name="o")
                                    nc.vector.tensor_scalar(
                                        o[:], iota_t[:],
                                        dl_sb[:, gcol:gcol + 1],
                                        vwl[:, gcol:gcol + 1],
                                        op0=mybir.AluOpType.is_equal,
                                        op1=mybir.AluOpType.mult)
                                    nc.tensor.matmul(
                                        st[:],
                                        lhsT=xg[:, col * P:(col + 1) * P],
                                        rhs=o[:],
                                        start=(ti == 0),
                                        stop=(ti == len(cols) - 1))
                                p2 = pq.tile([P, SB], F32, name="p2",
                                             space="PSUM")
                                xtb = xtbg[:, si * SB:(si + 1) * SB]
                                if cols:
                                    sn = sp.tile([P, SB], BF16, name="sn")
                                    if l == 3:
                                        nc.vector.tensor_copy(sn[:], st[:])
                                    else:
                                        nc.scalar.activation(
                                            sn[:], st[:],
                                            mybir.ActivationFunctionType.Copy)
                                    nc.tensor.matmul(p2[:], lhsT=Bs[:],
                                                     rhs=sn[:],
                                                     start=True, stop=False)
                                    nc.tensor.matmul(
                                        p2[:], lhsT=Rs[:], rhs=xtb,
                                        start=False, stop=True)
                                else:
                                    nc.tensor.matmul(
                                        p2[:], lhsT=Rs[:], rhs=xtb,
                                        start=True, stop=True)
                                outs = outg[:, si * SB:(si + 1) * SB]
                                nc.scalar.activation(
                                    outs, p2[:],
                                    mybir.ActivationFunctionType.Relu,
                                    bias=bs[:, :1])
                                if l < 3:
                                    for h in range(2):
                                        c0 = si * SB + h * P
                                        tp2 = pt.tile([P, P], BF16,
                                                      name="tp2",
                                                      space="PSUM")
                                        nc.tensor.transpose(
                                            tp2[:], outg[:, c0:c0 + P],
                                            ident[:])
                                        nc.scalar.activation(
                                            rowg[:, c0:c0 + P], tp2[:],
                                            mybir.ActivationFunctionType.Copy)
                                if is_var and s < NSB_VAR:
                                    for h in range(2):
                                        cc0 = si * SB + h * P
                                        hp = pt.tile([P, P], F32, name="hp",
                                                     space="PSUM")
                                        for hl in range(3):
                                            nc.tensor.matmul(
                                                hp[:], lhsT=fc1c[hl][:],
                                                rhs=xtls[hl][:, cc0:cc0 + P],
                                                start=(hl == 0), stop=False)
                                        nc.tensor.matmul(
                                            hp[:], lhsT=fc1c[3][:],
                                            rhs=outg[:, cc0:cc0 + P],
                                            start=False, stop=True)
                                        hr = sp.tile([P, P], BF16, name="hr")
                                        nc.vector.tensor_scalar(
                                            hr[:], hp[:], fb1[:, :1], 0.0,
                                            op0=mybir.AluOpType.add,
                                            op1=mybir.AluOpType.max)
                                        yp = py.tile([P, 1], F32, name="yp",
                                                     space="PSUM")
                                        nc.tensor.matmul(yp[:], lhsT=hr[:],
                                                         rhs=f4w[:],
                                                         start=True,
                                                         stop=True)
                                        nc.vector.tensor_scalar(
                                            ybuf[:, si * 2 + h:si * 2 + h + 1],
                                            yp[:], f4b[:, :1], None,
                                            op0=mybir.AluOpType.add)
                            nc.sync.dma_start(
                                xTn[:, s_base * SB:(s_base + sub_n) * SB],
                                outg[:, :sub_n * SB])
                            if l < 3:
                                r0 = s_base * SB
                                nc.sync.dma_start(
                                    ag_in[l - 1][r0:r0 + sub_n * SB, :]
                                    .rearrange("(t p) d -> p t d", p=P),
                                    rowg[:, :sub_n * SB].rearrange(
                                        "p (t d) -> p t d", d=D))
                            if is_var:
                                nc.sync.dma_start(
                                    y_out[s_base * SB:(s_base + gnv) * SB]
                                    .rearrange("(g p) -> p g", p=P),
                                    ybuf[:, :gnv * 2])
                            sub0 += sub_n
                            if l < 3:
                                rows_done = (s_base + sub_n) * SB
                                while (ly_pieces < len(PIECES)
                                       and rows_done
                                       >= (PIECES[ly_pieces][0]
                                           + PIECES[ly_pieces][1])):
                                    pz = ly_pieces
                                    r0, rp = PIECES[pz]
                                    nc.gpsimd.collective_compute(
                                        "AllGather", mybir.AluOpType.bypass,
                                        replica_groups=rgroups,
                                        ins=[ag_in[l - 1][r0:r0 + rp, :]],
                                        outs=[x_p[l][pz][:]])
                                    ly_pieces += 1


    nc.compile()
    return nc


_CACHE = {}


def kernel(**inputs) -> np.ndarray:
    from concourse import bass_utils
    in_maps, tpb2, groups, call_tiles, T2, var_slot, key = _preprocess(inputs)
    if key not in _CACHE:
        _CACHE[key] = _build_program(tpb2, groups, call_tiles, T2)
    nc = _CACHE[key]
    res = bass_utils.run_bass_kernel_spmd(
        nc, in_maps, core_ids=list(range(N_CORES)))
    y = np.empty(N_VAR, np.float32)
    for c in range(N_CORES):
        ys = res.results[c]["y_out"]
        vids = np.arange(c * NV_CORE, (c + 1) * NV_CORE)
        y[vids] = ys[var_slot[vids]]
    return y.astype(np.float32)
